# revision 28
# baseline (speedup 1.0000x reference)
"""DND retrieval (episodic memory read) kernel for 8 Trainium2 NeuronCores.

Data-parallel over batch B=64 -> 8 envs per core, with step-aware
packing: only ceil(step/128) l-chunks per env are ever touched (the
rest are masked to zero by the softmax validity mask), so the host
packs exactly those chunks, assigns envs to cores by sorted rank so
every core shares one compiled chunk pattern C*, and the kernel skips
the dead ~45% of keys/vals DMA and PE work.

Precision: keys (with rpe * 64/sqrt(K) folded in) and the q-side MLP
stream as fp8e4m3 (weights x32, qc x32, q x16 host/chip scales); the
scores and Wq matmuls run in fp8 DoubleRow mode (2 contraction rows
per partition, 2x PE rate). vals and output-side weights stay bf16
(fp8 there pushes error past budget).

Scores are processed in 512-column windows of the packed image through
a 2-bank PSUM ring: scores -> exp(S/1024) -> multiply by a precomputed
validity mask -> unnormalized probs transpose straight into the value
matmul; softmax 1/Z is applied to the [64, 512] result instead
(linearity), so nothing waits on the global sum. Scores are tiny
(|s| < 0.3), so no max pass is needed.
"""
from contextlib import ExitStack

import numpy as np
import ml_dtypes

import concourse.bass as bass
import concourse.tile as tile
from concourse import bacc, mybir
from concourse.bass_utils import run_bass_kernel_spmd
from concourse.masks import make_identity

F32 = mybir.dt.float32
BF16 = mybir.dt.bfloat16
FP8 = mybir.dt.float8e4
AF = mybir.ActivationFunctionType
OP = mybir.AluOpType
DR = mybir.MatmulPerfMode.DoubleRow

L = 1024
B = 64        # rows of the batched softmax image: (slot, head)
BL = 8        # envs (slots) per core
KD = 512
VD = 512
H = 8
MEMB = 256
SDIM = 512
HID = 512
RIMQ = 512
LAT = KD - MEMB
NCORES = 8
KC = KD // 128
RSQK = 1.0 / np.sqrt(np.float32(KD))
KSCALE = 64.0          # folded into keys on host
WSCALE = 32.0          # fp8 weight scale
QCS = 32.0             # qc activation fp8 scale
QS = 16.0              # q fp8 scale inside Qpad
NBF16 = np.dtype(ml_dtypes.bfloat16)
NFP8 = np.dtype(ml_dtypes.float8_e4m3)
SEQ = [0, 7, 1, 6, 2, 5, 3, 4]   # packed slot order

_CACHE: dict = {}


def _emit(nc: bass.Bass, tc: tile.TileContext, ctx: ExitStack, io: dict,
          cstar: tuple):
    # ---- packed geometry (compile-time) ----
    seqc = [cstar[s] for s in SEQ]
    offs = np.concatenate([[0], np.cumsum(seqc)])
    NCH = int(offs[-1])
    W = NCH * 128
    owner = []                       # chunk idx -> slot
    for p, s in enumerate(SEQ):
        owner += [s] * seqc[p]
    NW = (NCH + 3) // 4              # 512-col score windows
    NS = (NW + 1) // 2               # keys DMA slabs (2 windows each)

    pool = ctx.enter_context(tc.tile_pool(name="main", bufs=1))
    kpool = ctx.enter_context(tc.tile_pool(name="keys", bufs=2 * NS))
    ebpool = ctx.enter_context(tc.tile_pool(name="eb", bufs=2))
    wpool = ctx.enter_context(tc.tile_pool(name="wstream", bufs=2))
    evpool = ctx.enter_context(tc.tile_pool(name="evt", bufs=6))
    psum = ctx.enter_context(tc.tile_pool(name="ps", bufs=2, space="PSUM"))
    spsum = ctx.enter_context(tc.tile_pool(name="ps2", bufs=2, space="PSUM"))
    rpsum = ctx.enter_context(tc.tile_pool(name="ps3", bufs=1, space="PSUM"))
    scps = ctx.enter_context(tc.tile_pool(name="ps4", bufs=2, space="PSUM"))

    identb = pool.tile([128, 128], BF16)
    make_identity(nc, identb[:])

    def bias_tile(name, nch, eng=None):
        t = pool.tile([128, nch], F32, tag="b" + name)
        (eng or nc.sync).dma_start(t[:], io[name][:])
        return t

    # ---------------- Phase A: q-side MLP (fp8, DoubleRow Wq) -------------
    stateT_n = pool.tile([128, SDIM // 128, BL], FP8)
    nc.sync.dma_start(stateT_n[:], io["stateT"][:])
    latT_n = pool.tile([128, LAT // 128, BL], BF16)
    nc.sync.dma_start(latT_n[:], io["latT"][:])

    bst = bias_tile("b_state", 2)        # x32
    bcq1 = bias_tile("bcq1", 4)          # x32
    bcq2 = bias_tile("bcq2", 4)          # x32
    bq = bias_tile("bq", 32)             # x(32*QCS)

    stateT = [stateT_n[:, c, :] for c in range(SDIM // 128)]
    latT = [latT_n[:, c, :] for c in range(LAT // 128)]

    def layer_T(xT_chunks, w_name, b_tile, n_out, tag, wdt=BF16, scale=None,
                out_dt=BF16, eng=None):
        nk = len(xT_chunks)
        w = wpool.tile([128, nk, n_out], wdt,
                       tag="Wstg8" if wdt == FP8 else "Wstgb")
        (eng or nc.sync).dma_start(w[:], io[w_name][:])
        outs = []
        for j in range(n_out // 128):
            ps = psum.tile([128, BL], F32, tag="sm")
            for k in range(nk):
                nc.tensor.matmul(ps[:], w[:, k, j * 128:(j + 1) * 128],
                                 xT_chunks[k], start=(k == 0),
                                 stop=(k == nk - 1), skip_group_check=True)
            t = pool.tile([128, BL], out_dt, tag=f"{tag}{j}")
            if scale is None:
                nc.vector.tensor_scalar(out=t[:], in0=ps[:],
                                        scalar1=b_tile[:, j:j + 1],
                                        scalar2=None, op0=OP.add)
            else:
                nc.vector.tensor_scalar(out=t[:], in0=ps[:],
                                        scalar1=b_tile[:, j:j + 1],
                                        scalar2=scale, op0=OP.add,
                                        op1=OP.mult)
            outs.append(t[:])
        return outs

    RW = 1.0 / WSCALE
    xT = layer_T(stateT, "W_state", bst, MEMB, "xT", wdt=FP8, scale=RW) + latT
    h1T = layer_T(xT, "Wcq1", bcq1, HID, "h1", wdt=FP8, scale=RW,
                  eng=nc.scalar)
    # qc layer -> single fp8 tile (x QCS), consumed as DoubleRow lhsT.
    # Padded to QCW columns: dual-fp8 LDWEIGHTS rejects 8-wide loads.
    QCW = 32
    qcT = pool.tile([128, KC, QCW], FP8)
    nc.gpsimd.memset(qcT[:], 0.0)
    wcq2 = wpool.tile([128, KC, KD], FP8, tag="Wstg8")
    nc.sync.dma_start(wcq2[:], io["Wcq2"][:])
    for j in range(KC):
        ps = psum.tile([128, BL], F32, tag="sm")
        for k in range(KC):
            nc.tensor.matmul(ps[:], wcq2[:, k, j * 128:(j + 1) * 128],
                             h1T[k], start=(k == 0), stop=(k == KC - 1),
                             skip_group_check=True)
        nc.vector.tensor_scalar(out=qcT[:, j, 0:BL], in0=ps[:],
                                scalar1=bcq2[:, j:j + 1], scalar2=QCS / 32.0,
                                op0=OP.add, op1=OP.mult)

    # Wq in DoubleRow fp8: out [8, 512] per (jg, kcp), then transpose and
    # scatter into Qpad (fp8, xQS) diagonal windows.
    Qpad = pool.tile([128, 2, 2, BL, 72], FP8)
    nc.gpsimd.memset(Qpad[:], 0.0)
    wq = pool.tile([128, 2, 2, H * KD], FP8)
    for kcp in range(2):
        (nc.sync if kcp == 0 else nc.scalar).dma_start(
            wq[:, kcp, :, :], io["Wq"][:, kcp, :, :])
    QSC = QS / (32.0 * QCS)
    for jg in range(8):
        ps = spsum.tile([QCW, 512], F32, tag="sp")
        for kcp in range(2):
            nc.tensor.matmul(ps[:], qcT[:, 2 * kcp:2 * kcp + 2, :],
                             wq[:, kcp, :, jg * 512:(jg + 1) * 512],
                             start=(kcp == 0), stop=(kcp == 1),
                             perf_mode=DR, skip_group_check=True)
        qsb = pool.tile([BL, 512], BF16, tag="qsb")
        nc.scalar.copy(qsb[:], ps[0:BL, :])
        for jj in range(4):
            j = jg * 4 + jj
            h, kc = j // KC, j % KC
            tp = psum.tile([128, BL], BF16, tag="sm")
            nc.tensor.transpose(tp[:], qsb[:, jj * 128:(jj + 1) * 128],
                                identb[0:BL, 0:BL])
            nc.vector.tensor_scalar(
                out=Qpad[:, kc // 2, kc % 2, :, h], in0=tp[:],
                scalar1=bq[:, j:j + 1], scalar2=QSC, op0=OP.add, op1=OP.mult)
    qwin = [Qpad[:, kcp, :, :, :].rearrange("p i b c -> p i (b c)")
            for kcp in range(2)]

    # ---------------- validity mask, one full-width pass (early) ------------
    offT = pool.tile([B, 1], F32)
    nc.sync.dma_start(offT[:], io["offW"][:, 0:1])
    endT = pool.tile([B, 1], F32)
    nc.sync.dma_start(endT[:], io["endW"][:, 0:1])
    iot = pool.tile([B, W], F32)
    nc.gpsimd.iota(iot[:], pattern=[[1, W]], base=0, channel_multiplier=0,
                   allow_small_or_imprecise_dtypes=True)
    m1 = pool.tile([B, W], BF16)
    nc.vector.tensor_scalar(out=m1[:], in0=iot[:], scalar1=offT[:, 0:1],
                            scalar2=None, op0=OP.is_ge)
    valid = pool.tile([B, W], BF16)
    nc.vector.tensor_scalar(out=valid[:], in0=iot[:], scalar1=endT[:, 0:1],
                            scalar2=None, op0=OP.is_lt)
    nc.vector.tensor_tensor(out=valid[:], in0=valid[:], in1=m1[:],
                            op=OP.mult)

    # ---------------- keys slabs (fp8, DoubleRow layout) --------------------
    slabs = []          # (tile, chunk0, nchunks)
    for si in range(NS):
        c0, c1 = 8 * si, min(8 * si + 8, NCH)
        kts = []
        for kcp in range(2):
            kt = kpool.tile([128, 2, 1024], FP8, tag="kt")
            (nc.sync if kcp == 0 else nc.scalar).dma_start(
                kt[:, :, 0:(c1 - c0) * 128],
                io["keysT"][:, kcp, :, c0 * 128:c1 * 128])
            kts.append(kt)
        slabs.append((kts, c0, c1 - c0))

    # ---------------- vals + wagg streams (resident, after keys) ------------
    # greedy queue balancing; loads seeded with phase-A + keys bytes
    qload = {"sync": 2.3e6 + 64 * W, "scalar": 2.3e6 + 64 * W,
             "gpsimd": 0.2e6}
    qeng = {"sync": nc.sync, "scalar": nc.scalar, "gpsimd": nc.gpsimd}

    def pick(nbytes):
        q = min(qload, key=qload.get)
        qload[q] += nbytes
        return qeng[q]

    vres = pool.tile([128, NCH, VD], BF16)
    for p in range(BL):
        nch = int(offs[p + 1]) - int(offs[p])
        pick(nch * 131072).dma_start(
            vres[:, int(offs[p]):int(offs[p + 1]), :],
            io["vals"][:, int(offs[p]):int(offs[p + 1]), :])
    wagg = pool.tile([128, 32, VD], BF16)
    for gi in range(4):
        pick(1048576).dma_start(
            wagg[:, gi * 8:(gi + 1) * 8, :],
            io["Wagg"][:, gi * 8:(gi + 1) * 8, :])

    # ---------------- scores -> exp -> EV -> value matmul, pipelined --------
    EV = pool.tile([B, W], BF16)
    rps = rpsum.tile([B, VD], F32, tag="rp")

    sgs = [None] * NW

    def post(w):
        # exp -> mask-mult -> transpose -> value matmuls
        c0, c1 = 4 * w, min(4 * w + 4, NCH)
        gw = (c1 - c0) * 128
        eb = ebpool.tile([B, 512], BF16, tag="eb")
        nc.scalar.activation(eb[:, 0:gw], sgs[w][:, 0:gw], AF.Exp, bias=0.0,
                             scale=1.0 / (KSCALE * QS))
        nc.vector.tensor_tensor(out=EV[:, c0 * 128:c0 * 128 + gw],
                                in0=eb[:, 0:gw],
                                in1=valid[:, c0 * 128:c0 * 128 + gw],
                                op=OP.mult)
        for i in range(c0, c1):
            tpp = psum.tile([128, B], BF16, tag="sm")
            nc.tensor.transpose(tpp[:], EV[:, i * 128:(i + 1) * 128],
                                identb[0:B, 0:B])
            evt = evpool.tile([128, B], BF16, tag="evt")
            if i % 2 == 0:
                nc.scalar.copy(evt[:], tpp[:])
            else:
                nc.vector.tensor_copy(evt[:], tpp[:])
            nc.tensor.matmul(rps[:], evt[:], vres[:, i, :],
                             start=(i == 0), stop=(i == NCH - 1),
                             skip_group_check=True)

    for w in range(NW):
        c0, c1 = 4 * w, min(4 * w + 4, NCH)
        sg = scps.tile([B, 512], F32, tag="sg")
        sgs[w] = sg
        # matmul pieces: runs of chunks with the same owner slot
        i = c0
        while i < c1:
            j = i
            while j < c1 and owner[j] == owner[i]:
                j += 1
            s = owner[i]
            si, sc0 = i // 8, (i % 8) * 128
            lo, cw = (i - c0) * 128, (j - i) * 128
            kts = slabs[si][0]
            for kcp in range(2):
                nc.tensor.matmul(
                    sg[:, lo:lo + cw],
                    qwin[kcp][:, :, s * 64:s * 64 + 64],
                    kts[kcp][:, :, sc0:sc0 + cw],
                    start=(kcp == 0), stop=(kcp == 1),
                    perf_mode=DR, skip_group_check=True)
            i = j
        if w > 0:
            post(w - 1)
    post(NW - 1)

    # Z over the whole masked-prob image; R = 1/Z folded into the readout
    Zh = pool.tile([B, 1], F32)
    nc.vector.tensor_reduce(out=Zh[:], in_=EV[:], op=OP.add,
                            axis=mybir.AxisListType.X)
    R = pool.tile([B, 1], F32)
    nc.vector.reciprocal(R[:], Zh[:])
    rsb = pool.tile([B, VD], BF16, tag="rs")
    nc.vector.tensor_scalar(out=rsb[:], in0=rps[:], scalar1=R[:, 0:1],
                            scalar2=None, op0=OP.mult)
    RT = pool.tile([128, VD // 128, B], BF16)
    for vc in range(VD // 128):
        tr = psum.tile([128, B], BF16, tag="sm")
        nc.tensor.transpose(tr[:], rsb[:, vc * 128:(vc + 1) * 128],
                            identb[0:B, 0:B])
        nc.vector.tensor_copy(RT[:, vc, :], tr[:])

    # ---------------- Phase E: output MLP chain (bf16) ---------------------
    bagg = bias_tile("bagg", 4)
    brk1 = bias_tile("brk1", 4)
    brv1 = bias_tile("brv1", 4, eng=nc.scalar)

    aggp = spsum.tile([BL, VD], F32, tag="sp")
    for c in range(32):
        h, vc = c // 4, c % 4
        nc.tensor.matmul(aggp[:], RT[:, vc, h:B:H], wagg[:, c, :],
                         start=(c == 0), stop=(c == 31),
                         skip_group_check=True)
    aggsb = pool.tile([BL, VD], BF16, tag="aggsb")
    nc.scalar.copy(aggsb[:], aggp[:])
    AT = []
    for j in range(VD // 128):
        tp = psum.tile([128, BL], BF16, tag="sm")
        nc.tensor.transpose(tp[:], aggsb[:, j * 128:(j + 1) * 128],
                            identb[0:BL, 0:BL])
        t = pool.tile([128, BL], BF16, tag=f"AT{j}")
        nc.vector.tensor_scalar(out=t[:], in0=tp[:],
                                scalar1=bagg[:, j:j + 1],
                                scalar2=None, op0=OP.add)
        AT.append(t[:])

    ones = pool.tile([1, BL], F32)
    nc.gpsimd.memset(ones[:], 1.0)

    def bias_bcast(name, eng=None):
        brow = pool.tile([1, 512], F32, tag="br" + name)
        (eng or nc.sync).dma_start(brow[:], io[name][:])
        bb = spsum.tile([BL, 512], F32, tag="sp")
        nc.tensor.matmul(bb[:], ones[:], brow[:], start=True, stop=True)
        bsb = pool.tile([BL, 512], F32, tag="bs" + name)
        nc.vector.tensor_copy(bsb[:], bb[:])
        return bsb

    bk2 = bias_bcast("brk2_flat")
    bv2 = bias_bcast("brv2_flat", eng=nc.scalar)

    def layer_nat(xT_chunks, w_name, n_out, eng=None):
        nk = len(xT_chunks)
        w = wpool.tile([128, nk, n_out], BF16, tag="Wstgb")
        (eng or nc.sync).dma_start(w[:], io[w_name][:])
        ps = spsum.tile([BL, n_out], F32, tag="sp")
        for k in range(nk):
            nc.tensor.matmul(ps[:], xT_chunks[k], w[:, k, :],
                             start=(k == 0), stop=(k == nk - 1),
                             skip_group_check=True)
        return ps

    hkT = layer_T(AT, "Wrk1", brk1, HID, "hk", eng=pick(524288))
    ok_ps = layer_nat(hkT, "Wrk2", RIMQ, eng=pick(524288))
    hvT = layer_T(AT, "Wrv1", brv1, HID, "hv", eng=pick(524288))
    ov_ps = layer_nat(hvT, "Wrv2", VD, eng=pick(524288))

    for name, ps_, bias_sb in (("out_key", ok_ps, bk2), ("out_val", ov_ps, bv2)):
        onat = pool.tile([BL, 512], F32, tag="o" + name)
        nc.vector.tensor_tensor(out=onat[:], in0=ps_[:], in1=bias_sb[:],
                                op=OP.add)
        nc.sync.dma_start(io[name][:], onat[:])


def _build(cstar):
    seqc = [cstar[s] for s in SEQ]
    NCH = int(sum(seqc))
    W = NCH * 128
    NW = (NCH + 3) // 4
    nc = bacc.Bacc("TRN2", target_bir_lowering=False, debug=False,
                   num_devices=NCORES)
    io = {}

    def din(name, shape, dt=BF16):
        io[name] = nc.dram_tensor(name, shape, dt, kind="ExternalInput").ap()

    din("keysT", [128, 2, 2, W], FP8)
    din("vals", [128, NCH, VD])
    din("offW", [B, NW], F32)
    din("endW", [B, NW], F32)
    din("stateT", [128, SDIM // 128, BL], FP8)
    din("latT", [128, LAT // 128, BL])
    din("W_state", [128, KC, MEMB], FP8)
    din("b_state", [128, 2], F32)
    din("Wcq1", [128, KC, HID], FP8)
    din("bcq1", [128, 4], F32)
    din("Wcq2", [128, KC, KD], FP8)
    din("bcq2", [128, 4], F32)
    din("Wq", [128, 2, 2, H * KD], FP8)
    din("bq", [128, 32], F32)
    din("Wagg", [128, 32, VD])
    din("bagg", [128, 4], F32)
    din("Wrk1", [128, KC, HID])
    din("brk1", [128, 4], F32)
    din("Wrk2", [128, KC, RIMQ])
    din("brk2_flat", [1, 512], F32)
    din("Wrv1", [128, KC, HID])
    din("brv1", [128, 4], F32)
    din("Wrv2", [128, KC, VD])
    din("brv2_flat", [1, 512], F32)
    io["out_key"] = nc.dram_tensor("out_key", [BL, RIMQ], F32,
                                   kind="ExternalOutput").ap()
    io["out_val"] = nc.dram_tensor("out_val", [BL, VD], F32,
                                   kind="ExternalOutput").ap()

    with tile.TileContext(nc) as tc, ExitStack() as ctx:
        _emit(nc, tc, ctx, io, cstar)
    nc.compile()
    return nc


def _rsb(bias, nch, scale=1.0):
    return np.ascontiguousarray(
        np.asarray(bias, np.float32).reshape(nch, 128).T * scale)


def _wchunk(w, dt=NBF16, scale=1.0):
    w = np.asarray(w, np.float32) * scale
    f, c = w.shape
    return np.ascontiguousarray(
        w.reshape(f // 128, 128, c).transpose(1, 0, 2)).astype(dt)


def _actT(x, dt):
    x = np.asarray(x, np.float32)
    bl, f = x.shape
    return np.ascontiguousarray(
        x.T.reshape(f // 128, 128, bl).transpose(1, 0, 2)).astype(dt)


def _plan(step):
    cb = np.clip((np.asarray(step, np.int64) + 127) // 128, 1, 8)
    order = np.argsort(-cb, kind="stable")
    cstar = tuple(int(cb[order[8 * s]]) for s in range(BL))
    return order, cstar


def _shard(inputs):
    f = lambda x: np.asarray(x, np.float32)
    keys, vals, rpe = f(inputs["keys"]), f(inputs["vals"]), f(inputs["rpe_mod"])
    step = np.asarray(inputs["step"]).astype(np.int64)
    state, lat = f(inputs["state"]), f(inputs["task_inference_latent"])

    order, cstar = _plan(step)
    seqc = [cstar[s] for s in SEQ]
    offs = np.concatenate([[0], np.cumsum(seqc)])
    NCH = int(offs[-1])
    NW = (NCH + 3) // 4

    shared = {
        "W_state": _wchunk(inputs["W_state"], NFP8, WSCALE),
        "b_state": _rsb(inputs["b_state"], 2, WSCALE),
        "Wcq1": _wchunk(inputs["Wcq1"], NFP8, WSCALE),
        "bcq1": _rsb(inputs["bcq1"], 4, WSCALE),
        "Wcq2": _wchunk(inputs["Wcq2"], NFP8, WSCALE),
        "bcq2": _rsb(inputs["bcq2"], 4, WSCALE),
        "Wq": _wchunk(inputs["Wq"], NFP8, WSCALE).reshape(128, 2, 2, H * KD),
        "bq": _rsb(inputs["bq"], 32, WSCALE * QCS),
        "Wagg": _wchunk(inputs["Wagg"]),
        "bagg": _rsb(inputs["bagg"], 4),
        "Wrk1": _wchunk(inputs["Wrk1"]), "brk1": _rsb(inputs["brk1"], 4),
        "Wrk2": _wchunk(inputs["Wrk2"]),
        "brk2_flat": np.ascontiguousarray(f(inputs["brk2"])[None, :]),
        "Wrv1": _wchunk(inputs["Wrv1"]), "brv1": _rsb(inputs["brv1"], 4),
        "Wrv2": _wchunk(inputs["Wrv2"]),
        "brv2_flat": np.ascontiguousarray(f(inputs["brv2"])[None, :]),
    }
    kfold = keys * rpe * (KSCALE * RSQK)            # [L, 64, K]
    in_maps = []
    for m in range(NCORES):
        envs = [int(order[8 * s + m]) for s in range(BL)]
        kp = np.zeros((128, 2, 2, NCH * 128), NFP8)
        vp = np.zeros((128, NCH, VD), NBF16)
        offW = np.zeros((B, NW), np.float32)
        endW = np.zeros((B, NW), np.float32)
        for p, s in enumerate(SEQ):
            e = envs[s]
            nl = cstar[s] * 128
            c0, c1 = int(offs[p]), int(offs[p + 1])
            kb = kfold[:nl, e, :].T.reshape(2, 2, 128, nl).transpose(
                2, 0, 1, 3)
            kp[:, :, :, c0 * 128:c1 * 128] = kb.astype(NFP8)
            vb = vals[:nl, e, :].reshape(cstar[s], 128, VD).transpose(1, 0, 2)
            vp[:, c0:c1, :] = vb.astype(NBF16)
            for w in range(NW):
                offW[s * H:(s + 1) * H, w] = c0 * 128 - w * 512
                endW[s * H:(s + 1) * H, w] = (c0 * 128 - w * 512
                                              + float(step[e]))
        in_maps.append({
            "keysT": kp, "vals": vp, "offW": offW, "endW": endW,
            "stateT": _actT(state[envs], NFP8),
            "latT": _actT(lat[envs], NBF16),
            **shared,
        })
    return in_maps, order


def kernel(**inputs):
    order, cstar = _plan(inputs["step"])
    nc = _CACHE.get(cstar)
    if nc is None:
        nc = _CACHE[cstar] = _build(cstar)
    in_maps, order = _shard(inputs)
    res = run_bass_kernel_spmd(nc, in_maps, list(range(NCORES)),
                               **_CACHE.get("run_kwargs", {}))
    _CACHE["last_result"] = res
    ok = np.empty((B, RIMQ), np.float32)
    ov = np.empty((B, VD), np.float32)
    for m in range(NCORES):
        for s in range(BL):
            e = int(order[8 * s + m])
            ok[e] = res.results[m]["out_key"][s]
            ov[e] = res.results[m]["out_val"][s]
    return ok[:, None, :], ov[:, None, :]


# revision 29
# speedup vs baseline: 1.0827x; 1.0827x over previous
"""DND retrieval (episodic memory read) kernel for 8 Trainium2 NeuronCores.

Data-parallel over batch B=64 -> 8 envs per core, with step-aware
packing: only ceil(step/128) l-chunks per env are ever touched (the
rest are masked to zero by the softmax validity mask), so the host
packs exactly those chunks, assigns envs to cores by sorted rank so
every core shares one compiled chunk pattern C*, and the kernel skips
the dead ~45% of keys/vals DMA and PE work.

Precision: keys (with rpe * 64/sqrt(K) folded in) and the q-side MLP
stream as fp8e4m3 (weights x32, qc x32, q x16 host/chip scales); the
scores and Wq matmuls run in fp8 DoubleRow mode (2 contraction rows
per partition, 2x PE rate). vals and output-side weights stay bf16
(fp8 there pushes error past budget).

Scores are processed in 512-column windows of the packed image through
a 2-bank PSUM ring: scores -> exp(S/1024) -> multiply by a precomputed
validity mask -> unnormalized probs transpose straight into the value
matmul; softmax 1/Z is applied to the [64, 512] result instead
(linearity), so nothing waits on the global sum. Scores are tiny
(|s| < 0.3), so no max pass is needed.
"""
from contextlib import ExitStack

import numpy as np
import ml_dtypes

import concourse.bass as bass
import concourse.tile as tile
from concourse import bacc, mybir
from concourse.bass_utils import run_bass_kernel_spmd
from concourse.masks import make_identity

F32 = mybir.dt.float32
BF16 = mybir.dt.bfloat16
FP8 = mybir.dt.float8e4
AF = mybir.ActivationFunctionType
OP = mybir.AluOpType
DR = mybir.MatmulPerfMode.DoubleRow

L = 1024
B = 64        # rows of the batched softmax image: (slot, head)
BL = 8        # envs (slots) per core
KD = 512
VD = 512
H = 8
MEMB = 256
SDIM = 512
HID = 512
RIMQ = 512
LAT = KD - MEMB
NCORES = 8
KC = KD // 128
RSQK = 1.0 / np.sqrt(np.float32(KD))
KSCALE = 64.0          # folded into keys on host
WSCALE = 32.0          # fp8 weight scale
QCS = 32.0             # qc activation fp8 scale
QS = 16.0              # q fp8 scale inside Qpad
NBF16 = np.dtype(ml_dtypes.bfloat16)
NFP8 = np.dtype(ml_dtypes.float8_e4m3)
SEQ = [0, 7, 1, 6, 2, 5, 3, 4]   # packed slot order

_CACHE: dict = {}


def _emit(nc: bass.Bass, tc: tile.TileContext, ctx: ExitStack, io: dict,
          cstar: tuple):
    # ---- packed geometry (compile-time) ----
    seqc = [cstar[s] for s in SEQ]
    offs = np.concatenate([[0], np.cumsum(seqc)])
    NCH = int(offs[-1])
    W = NCH * 128
    owner = []                       # chunk idx -> slot
    for p, s in enumerate(SEQ):
        owner += [s] * seqc[p]
    NW = (NCH + 3) // 4              # 512-col score windows
    NS = (NW + 1) // 2               # keys DMA slabs (2 windows each)

    pool = ctx.enter_context(tc.tile_pool(name="main", bufs=1))
    kpool = ctx.enter_context(tc.tile_pool(name="keys", bufs=2 * NS))
    ebpool = ctx.enter_context(tc.tile_pool(name="eb", bufs=2))
    wpool = ctx.enter_context(tc.tile_pool(name="wstream", bufs=2))
    evpool = ctx.enter_context(tc.tile_pool(name="evt", bufs=6))
    psum = ctx.enter_context(tc.tile_pool(name="ps", bufs=2, space="PSUM"))
    spsum = ctx.enter_context(tc.tile_pool(name="ps2", bufs=2, space="PSUM"))
    rpsum = ctx.enter_context(tc.tile_pool(name="ps3", bufs=1, space="PSUM"))
    scps = ctx.enter_context(tc.tile_pool(name="ps4", bufs=3, space="PSUM"))

    identb = pool.tile([128, 128], BF16)
    make_identity(nc, identb[:])

    def bias_tile(name, nch, eng=None):
        t = pool.tile([128, nch], F32, tag="b" + name)
        (eng or nc.sync).dma_start(t[:], io[name][:])
        return t

    # ---------------- validity mask, one full-width pass (early) ------------
    offT = pool.tile([B, 1], F32)
    nc.sync.dma_start(offT[:], io["offW"][:, 0:1])
    endT = pool.tile([B, 1], F32)
    nc.sync.dma_start(endT[:], io["endW"][:, 0:1])
    iot = pool.tile([B, W], F32)
    nc.gpsimd.iota(iot[:], pattern=[[1, W]], base=0, channel_multiplier=0,
                   allow_small_or_imprecise_dtypes=True)
    m1 = pool.tile([B, W], BF16)
    nc.vector.tensor_scalar(out=m1[:], in0=iot[:], scalar1=offT[:, 0:1],
                            scalar2=None, op0=OP.is_ge)
    valid = pool.tile([B, W], BF16)
    nc.vector.tensor_scalar(out=valid[:], in0=iot[:], scalar1=endT[:, 0:1],
                            scalar2=None, op0=OP.is_lt)
    nc.vector.tensor_tensor(out=valid[:], in0=valid[:], in1=m1[:],
                            op=OP.mult)

    # ---------------- Phase A: q-side MLP (fp8, DoubleRow Wq) -------------
    stateT_n = pool.tile([128, SDIM // 128, BL], FP8)
    nc.sync.dma_start(stateT_n[:], io["stateT"][:])
    latT_n = pool.tile([128, LAT // 128, BL], BF16)
    nc.sync.dma_start(latT_n[:], io["latT"][:])

    bst = bias_tile("b_state", 2)        # x32
    bcq1 = bias_tile("bcq1", 4)          # x32
    bcq2 = bias_tile("bcq2", 4)          # x32
    bq = bias_tile("bq", 32)             # x(32*QCS)

    stateT = [stateT_n[:, c, :] for c in range(SDIM // 128)]
    latT = [latT_n[:, c, :] for c in range(LAT // 128)]

    def layer_T(xT_chunks, w_name, b_tile, n_out, tag, wdt=BF16, scale=None,
                out_dt=BF16, eng=None):
        nk = len(xT_chunks)
        w = wpool.tile([128, nk, n_out], wdt,
                       tag="Wstg8" if wdt == FP8 else "Wstgb")
        (eng or nc.sync).dma_start(w[:], io[w_name][:])
        outs = []
        for j in range(n_out // 128):
            ps = psum.tile([128, BL], F32, tag="sm")
            for k in range(nk):
                nc.tensor.matmul(ps[:], w[:, k, j * 128:(j + 1) * 128],
                                 xT_chunks[k], start=(k == 0),
                                 stop=(k == nk - 1), skip_group_check=True)
            t = pool.tile([128, BL], out_dt, tag=f"{tag}{j}")
            if scale is None:
                nc.vector.tensor_scalar(out=t[:], in0=ps[:],
                                        scalar1=b_tile[:, j:j + 1],
                                        scalar2=None, op0=OP.add)
            else:
                nc.vector.tensor_scalar(out=t[:], in0=ps[:],
                                        scalar1=b_tile[:, j:j + 1],
                                        scalar2=scale, op0=OP.add,
                                        op1=OP.mult)
            outs.append(t[:])
        return outs

    RW = 1.0 / WSCALE
    xT = layer_T(stateT, "W_state", bst, MEMB, "xT", wdt=FP8, scale=RW) + latT
    h1T = layer_T(xT, "Wcq1", bcq1, HID, "h1", wdt=FP8, scale=RW,
                  eng=nc.scalar)
    # qc layer -> single fp8 tile (x QCS), consumed as DoubleRow lhsT.
    # Padded to QCW columns: dual-fp8 LDWEIGHTS rejects 8-wide loads.
    QCW = 32
    qcT = pool.tile([128, KC, QCW], FP8)
    nc.gpsimd.memset(qcT[:], 0.0)
    wcq2 = wpool.tile([128, KC, KD], FP8, tag="Wstg8")
    nc.sync.dma_start(wcq2[:], io["Wcq2"][:])
    for j in range(KC):
        ps = psum.tile([128, BL], F32, tag="sm")
        for k in range(KC):
            nc.tensor.matmul(ps[:], wcq2[:, k, j * 128:(j + 1) * 128],
                             h1T[k], start=(k == 0), stop=(k == KC - 1),
                             skip_group_check=True)
        nc.vector.tensor_scalar(out=qcT[:, j, 0:BL], in0=ps[:],
                                scalar1=bcq2[:, j:j + 1], scalar2=QCS / 32.0,
                                op0=OP.add, op1=OP.mult)

    # Wq in DoubleRow fp8: out [8, 512] per (jg, kcp), then transpose and
    # scatter into Qpad (fp8, xQS) diagonal windows.
    Qpad = pool.tile([128, 2, 2, BL, 72], FP8)
    nc.gpsimd.memset(Qpad[:], 0.0)
    wq = pool.tile([128, 2, 2, H * KD], FP8)
    for kcp in range(2):
        (nc.sync if kcp == 0 else nc.scalar).dma_start(
            wq[:, kcp, :, :], io["Wq"][:, kcp, :, :])
    QSC = QS / (32.0 * QCS)
    for jg in range(8):
        ps = spsum.tile([QCW, 512], F32, tag="sp")
        for kcp in range(2):
            nc.tensor.matmul(ps[:], qcT[:, 2 * kcp:2 * kcp + 2, :],
                             wq[:, kcp, :, jg * 512:(jg + 1) * 512],
                             start=(kcp == 0), stop=(kcp == 1),
                             perf_mode=DR, skip_group_check=True)
        qsb = pool.tile([BL, 512], BF16, tag="qsb")
        nc.scalar.copy(qsb[:], ps[0:BL, :])
        for jj in range(4):
            j = jg * 4 + jj
            h, kc = j // KC, j % KC
            tp = psum.tile([128, BL], BF16, tag="sm")
            nc.tensor.transpose(tp[:], qsb[:, jj * 128:(jj + 1) * 128],
                                identb[0:BL, 0:BL])
            nc.vector.tensor_scalar(
                out=Qpad[:, kc // 2, kc % 2, :, h], in0=tp[:],
                scalar1=bq[:, j:j + 1], scalar2=QSC, op0=OP.add, op1=OP.mult)
    qwin = [Qpad[:, kcp, :, :, :].rearrange("p i b c -> p i (b c)")
            for kcp in range(2)]

    # ---------------- keys slabs (fp8, DoubleRow layout) --------------------
    slabs = []          # (tile, chunk0, nchunks)
    for si in range(NS):
        c0, c1 = 8 * si, min(8 * si + 8, NCH)
        kts = []
        for kcp in range(2):
            kt = kpool.tile([128, 2, 1024], FP8, tag="kt")
            (nc.sync if kcp == 0 else nc.scalar).dma_start(
                kt[:, :, 0:(c1 - c0) * 128],
                io["keysT"][:, kcp, :, c0 * 128:c1 * 128])
            kts.append(kt)
        slabs.append((kts, c0, c1 - c0))

    # ---------------- vals + wagg streams (resident, after keys) ------------
    # greedy queue balancing; loads seeded with phase-A + keys bytes
    qload = {"sync": 2.3e6 + 64 * W, "scalar": 2.3e6 + 64 * W,
             "gpsimd": 0.9e6}
    qeng = {"sync": nc.sync, "scalar": nc.scalar, "gpsimd": nc.gpsimd}

    def pick(nbytes):
        q = min(qload, key=qload.get)
        qload[q] += nbytes
        return qeng[q]

    vres = pool.tile([128, NCH, VD], BF16)
    for p in range(BL):
        nch = int(offs[p + 1]) - int(offs[p])
        pick(nch * 131072).dma_start(
            vres[:, int(offs[p]):int(offs[p + 1]), :],
            io["vals"][:, int(offs[p]):int(offs[p + 1]), :])
    wagg = pool.tile([128, 32, VD], BF16)
    for gi in range(4):
        pick(1048576).dma_start(
            wagg[:, gi * 8:(gi + 1) * 8, :],
            io["Wagg"][:, gi * 8:(gi + 1) * 8, :])

    # ---------------- scores -> exp -> EV -> value matmul, pipelined --------
    EV = pool.tile([B, W], BF16)
    rps = rpsum.tile([B, VD], F32, tag="rp")

    sgs = [None] * NW

    def post(w):
        # exp -> mask-mult -> transpose -> value matmuls
        c0, c1 = 4 * w, min(4 * w + 4, NCH)
        gw = (c1 - c0) * 128
        eb = ebpool.tile([B, 512], BF16, tag="eb")
        nc.scalar.activation(eb[:, 0:gw], sgs[w][:, 0:gw], AF.Exp, bias=0.0,
                             scale=1.0 / (KSCALE * QS))
        nc.vector.tensor_tensor(out=EV[:, c0 * 128:c0 * 128 + gw],
                                in0=eb[:, 0:gw],
                                in1=valid[:, c0 * 128:c0 * 128 + gw],
                                op=OP.mult)
        for i in range(c0, c1):
            tpp = psum.tile([128, B], BF16, tag="sm")
            nc.tensor.transpose(tpp[:], EV[:, i * 128:(i + 1) * 128],
                                identb[0:B, 0:B])
            evt = evpool.tile([128, B], BF16, tag="evt")
            if i % 2 == 0:
                nc.scalar.copy(evt[:], tpp[:])
            else:
                nc.vector.tensor_copy(evt[:], tpp[:])
            nc.tensor.matmul(rps[:], evt[:], vres[:, i, :],
                             start=(i == 0), stop=(i == NCH - 1),
                             skip_group_check=True)

    for w in range(NW):
        c0, c1 = 4 * w, min(4 * w + 4, NCH)
        sg = scps.tile([B, 512], F32, tag="sg")
        sgs[w] = sg
        # matmul pieces: runs of chunks with the same owner slot
        i = c0
        while i < c1:
            j = i
            while j < c1 and owner[j] == owner[i]:
                j += 1
            s = owner[i]
            si, sc0 = i // 8, (i % 8) * 128
            lo, cw = (i - c0) * 128, (j - i) * 128
            kts = slabs[si][0]
            for kcp in range(2):
                nc.tensor.matmul(
                    sg[:, lo:lo + cw],
                    qwin[kcp][:, :, s * 64:s * 64 + 64],
                    kts[kcp][:, :, sc0:sc0 + cw],
                    start=(kcp == 0), stop=(kcp == 1),
                    perf_mode=DR, skip_group_check=True)
            i = j
        if w > 1:
            post(w - 2)
    if NW > 1:
        post(NW - 2)
    post(NW - 1)

    # Z over the whole masked-prob image; R = 1/Z folded into the readout
    Zh = pool.tile([B, 1], F32)
    nc.vector.tensor_reduce(out=Zh[:], in_=EV[:], op=OP.add,
                            axis=mybir.AxisListType.X)
    R = pool.tile([B, 1], F32)
    nc.vector.reciprocal(R[:], Zh[:])
    rsb = pool.tile([B, VD], BF16, tag="rs")
    nc.vector.tensor_scalar(out=rsb[:], in0=rps[:], scalar1=R[:, 0:1],
                            scalar2=None, op0=OP.mult)
    RT = pool.tile([128, VD // 128, B], BF16)
    for vc in range(VD // 128):
        tr = psum.tile([128, B], BF16, tag="sm")
        nc.tensor.transpose(tr[:], rsb[:, vc * 128:(vc + 1) * 128],
                            identb[0:B, 0:B])
        nc.vector.tensor_copy(RT[:, vc, :], tr[:])

    # ---------------- Phase E: output MLP chain (bf16) ---------------------
    bagg = bias_tile("bagg", 4)
    brk1 = bias_tile("brk1", 4)
    brv1 = bias_tile("brv1", 4, eng=nc.scalar)

    aggp = spsum.tile([BL, VD], F32, tag="sp")
    for c in range(32):
        h, vc = c // 4, c % 4
        nc.tensor.matmul(aggp[:], RT[:, vc, h:B:H], wagg[:, c, :],
                         start=(c == 0), stop=(c == 31),
                         skip_group_check=True)
    aggsb = pool.tile([BL, VD], BF16, tag="aggsb")
    nc.scalar.copy(aggsb[:], aggp[:])
    AT = []
    for j in range(VD // 128):
        tp = psum.tile([128, BL], BF16, tag="sm")
        nc.tensor.transpose(tp[:], aggsb[:, j * 128:(j + 1) * 128],
                            identb[0:BL, 0:BL])
        t = pool.tile([128, BL], BF16, tag=f"AT{j}")
        nc.vector.tensor_scalar(out=t[:], in0=tp[:],
                                scalar1=bagg[:, j:j + 1],
                                scalar2=None, op0=OP.add)
        AT.append(t[:])

    ones = pool.tile([1, BL], F32)
    nc.gpsimd.memset(ones[:], 1.0)

    def bias_bcast(name, eng=None):
        brow = pool.tile([1, 512], F32, tag="br" + name)
        (eng or nc.sync).dma_start(brow[:], io[name][:])
        bb = spsum.tile([BL, 512], F32, tag="sp")
        nc.tensor.matmul(bb[:], ones[:], brow[:], start=True, stop=True)
        bsb = pool.tile([BL, 512], F32, tag="bs" + name)
        nc.vector.tensor_copy(bsb[:], bb[:])
        return bsb

    bk2 = bias_bcast("brk2_flat")
    bv2 = bias_bcast("brv2_flat", eng=nc.scalar)

    def layer_nat(xT_chunks, w_name, n_out, eng=None):
        nk = len(xT_chunks)
        w = wpool.tile([128, nk, n_out], BF16, tag="Wstgb")
        (eng or nc.sync).dma_start(w[:], io[w_name][:])
        ps = spsum.tile([BL, n_out], F32, tag="sp")
        for k in range(nk):
            nc.tensor.matmul(ps[:], xT_chunks[k], w[:, k, :],
                             start=(k == 0), stop=(k == nk - 1),
                             skip_group_check=True)
        return ps

    hkT = layer_T(AT, "Wrk1", brk1, HID, "hk", eng=pick(524288))
    ok_ps = layer_nat(hkT, "Wrk2", RIMQ, eng=pick(524288))
    hvT = layer_T(AT, "Wrv1", brv1, HID, "hv", eng=pick(524288))
    ov_ps = layer_nat(hvT, "Wrv2", VD, eng=pick(524288))

    for name, ps_, bias_sb in (("out_key", ok_ps, bk2), ("out_val", ov_ps, bv2)):
        onat = pool.tile([BL, 512], F32, tag="o" + name)
        nc.vector.tensor_tensor(out=onat[:], in0=ps_[:], in1=bias_sb[:],
                                op=OP.add)
        nc.sync.dma_start(io[name][:], onat[:])


def _build(cstar):
    seqc = [cstar[s] for s in SEQ]
    NCH = int(sum(seqc))
    W = NCH * 128
    NW = (NCH + 3) // 4
    nc = bacc.Bacc("TRN2", target_bir_lowering=False, debug=False,
                   num_devices=NCORES)
    io = {}

    def din(name, shape, dt=BF16):
        io[name] = nc.dram_tensor(name, shape, dt, kind="ExternalInput").ap()

    din("keysT", [128, 2, 2, W], FP8)
    din("vals", [128, NCH, VD])
    din("offW", [B, NW], F32)
    din("endW", [B, NW], F32)
    din("stateT", [128, SDIM // 128, BL], FP8)
    din("latT", [128, LAT // 128, BL])
    din("W_state", [128, KC, MEMB], FP8)
    din("b_state", [128, 2], F32)
    din("Wcq1", [128, KC, HID], FP8)
    din("bcq1", [128, 4], F32)
    din("Wcq2", [128, KC, KD], FP8)
    din("bcq2", [128, 4], F32)
    din("Wq", [128, 2, 2, H * KD], FP8)
    din("bq", [128, 32], F32)
    din("Wagg", [128, 32, VD])
    din("bagg", [128, 4], F32)
    din("Wrk1", [128, KC, HID])
    din("brk1", [128, 4], F32)
    din("Wrk2", [128, KC, RIMQ])
    din("brk2_flat", [1, 512], F32)
    din("Wrv1", [128, KC, HID])
    din("brv1", [128, 4], F32)
    din("Wrv2", [128, KC, VD])
    din("brv2_flat", [1, 512], F32)
    io["out_key"] = nc.dram_tensor("out_key", [BL, RIMQ], F32,
                                   kind="ExternalOutput").ap()
    io["out_val"] = nc.dram_tensor("out_val", [BL, VD], F32,
                                   kind="ExternalOutput").ap()

    with tile.TileContext(nc) as tc, ExitStack() as ctx:
        _emit(nc, tc, ctx, io, cstar)
    nc.compile()
    return nc


def _rsb(bias, nch, scale=1.0):
    return np.ascontiguousarray(
        np.asarray(bias, np.float32).reshape(nch, 128).T * scale)


def _wchunk(w, dt=NBF16, scale=1.0):
    w = np.asarray(w, np.float32) * scale
    f, c = w.shape
    return np.ascontiguousarray(
        w.reshape(f // 128, 128, c).transpose(1, 0, 2)).astype(dt)


def _actT(x, dt):
    x = np.asarray(x, np.float32)
    bl, f = x.shape
    return np.ascontiguousarray(
        x.T.reshape(f // 128, 128, bl).transpose(1, 0, 2)).astype(dt)


def _plan(step):
    cb = np.clip((np.asarray(step, np.int64) + 127) // 128, 1, 8)
    order = np.argsort(-cb, kind="stable")
    cstar = tuple(int(cb[order[8 * s]]) for s in range(BL))
    return order, cstar


def _shard(inputs):
    f = lambda x: np.asarray(x, np.float32)
    keys, vals, rpe = f(inputs["keys"]), f(inputs["vals"]), f(inputs["rpe_mod"])
    step = np.asarray(inputs["step"]).astype(np.int64)
    state, lat = f(inputs["state"]), f(inputs["task_inference_latent"])

    order, cstar = _plan(step)
    seqc = [cstar[s] for s in SEQ]
    offs = np.concatenate([[0], np.cumsum(seqc)])
    NCH = int(offs[-1])
    NW = (NCH + 3) // 4

    shared = {
        "W_state": _wchunk(inputs["W_state"], NFP8, WSCALE),
        "b_state": _rsb(inputs["b_state"], 2, WSCALE),
        "Wcq1": _wchunk(inputs["Wcq1"], NFP8, WSCALE),
        "bcq1": _rsb(inputs["bcq1"], 4, WSCALE),
        "Wcq2": _wchunk(inputs["Wcq2"], NFP8, WSCALE),
        "bcq2": _rsb(inputs["bcq2"], 4, WSCALE),
        "Wq": _wchunk(inputs["Wq"], NFP8, WSCALE).reshape(128, 2, 2, H * KD),
        "bq": _rsb(inputs["bq"], 32, WSCALE * QCS),
        "Wagg": _wchunk(inputs["Wagg"]),
        "bagg": _rsb(inputs["bagg"], 4),
        "Wrk1": _wchunk(inputs["Wrk1"]), "brk1": _rsb(inputs["brk1"], 4),
        "Wrk2": _wchunk(inputs["Wrk2"]),
        "brk2_flat": np.ascontiguousarray(f(inputs["brk2"])[None, :]),
        "Wrv1": _wchunk(inputs["Wrv1"]), "brv1": _rsb(inputs["brv1"], 4),
        "Wrv2": _wchunk(inputs["Wrv2"]),
        "brv2_flat": np.ascontiguousarray(f(inputs["brv2"])[None, :]),
    }
    kfold = keys * rpe * (KSCALE * RSQK)            # [L, 64, K]
    in_maps = []
    for m in range(NCORES):
        envs = [int(order[8 * s + m]) for s in range(BL)]
        kp = np.zeros((128, 2, 2, NCH * 128), NFP8)
        vp = np.zeros((128, NCH, VD), NBF16)
        offW = np.zeros((B, NW), np.float32)
        endW = np.zeros((B, NW), np.float32)
        for p, s in enumerate(SEQ):
            e = envs[s]
            nl = cstar[s] * 128
            c0, c1 = int(offs[p]), int(offs[p + 1])
            kb = kfold[:nl, e, :].T.reshape(2, 2, 128, nl).transpose(
                2, 0, 1, 3)
            kp[:, :, :, c0 * 128:c1 * 128] = kb.astype(NFP8)
            vb = vals[:nl, e, :].reshape(cstar[s], 128, VD).transpose(1, 0, 2)
            vp[:, c0:c1, :] = vb.astype(NBF16)
            for w in range(NW):
                offW[s * H:(s + 1) * H, w] = c0 * 128 - w * 512
                endW[s * H:(s + 1) * H, w] = (c0 * 128 - w * 512
                                              + float(step[e]))
        in_maps.append({
            "keysT": kp, "vals": vp, "offW": offW, "endW": endW,
            "stateT": _actT(state[envs], NFP8),
            "latT": _actT(lat[envs], NBF16),
            **shared,
        })
    return in_maps, order


def kernel(**inputs):
    order, cstar = _plan(inputs["step"])
    nc = _CACHE.get(cstar)
    if nc is None:
        nc = _CACHE[cstar] = _build(cstar)
    in_maps, order = _shard(inputs)
    res = run_bass_kernel_spmd(nc, in_maps, list(range(NCORES)),
                               **_CACHE.get("run_kwargs", {}))
    _CACHE["last_result"] = res
    ok = np.empty((B, RIMQ), np.float32)
    ov = np.empty((B, VD), np.float32)
    for m in range(NCORES):
        for s in range(BL):
            e = int(order[8 * s + m])
            ok[e] = res.results[m]["out_key"][s]
            ov[e] = res.results[m]["out_val"][s]
    return ok[:, None, :], ov[:, None, :]


# revision 30
# speedup vs baseline: 1.2113x; 1.1188x over previous
"""DND retrieval (episodic memory read) kernel for 8 Trainium2 NeuronCores.

Data-parallel over batch B=64 -> 8 envs per core, with step-aware
packing: only ceil(step/128) l-chunks per env are ever touched (the
rest are masked to zero by the softmax validity mask), so the host
packs exactly those chunks, assigns envs to cores by sorted rank so
every core shares one compiled chunk pattern C*, and the kernel skips
the dead ~45% of keys/vals DMA and PE work.

Precision: keys (with rpe * 64/sqrt(K) folded in) and the q-side MLP
stream as fp8e4m3 (weights x32, qc x32, q x16 host/chip scales); the
scores and Wq matmuls run in fp8 DoubleRow mode (2 contraction rows
per partition, 2x PE rate). vals and output-side weights stay bf16
(fp8 there pushes error past budget).

Scores are processed in 512-column windows of the packed image through
a 2-bank PSUM ring: scores -> exp(S/1024) -> multiply by a precomputed
validity mask -> unnormalized probs transpose straight into the value
matmul; softmax 1/Z is applied to the [64, 512] result instead
(linearity), so nothing waits on the global sum. Scores are tiny
(|s| < 0.3), so no max pass is needed.
"""
from contextlib import ExitStack

import numpy as np
import ml_dtypes

import concourse.bass as bass
import concourse.tile as tile
from concourse import bacc, mybir
from concourse.bass_utils import run_bass_kernel_spmd
from concourse.masks import make_identity

F32 = mybir.dt.float32
BF16 = mybir.dt.bfloat16
FP8 = mybir.dt.float8e4
AF = mybir.ActivationFunctionType
OP = mybir.AluOpType
DR = mybir.MatmulPerfMode.DoubleRow

L = 1024
B = 64        # rows of the batched softmax image: (slot, head)
BL = 8        # envs (slots) per core
KD = 512
VD = 512
H = 8
MEMB = 256
SDIM = 512
HID = 512
RIMQ = 512
LAT = KD - MEMB
NCORES = 8
KC = KD // 128
RSQK = 1.0 / np.sqrt(np.float32(KD))
KSCALE = 64.0          # folded into keys on host
WSCALE = 32.0          # fp8 weight scale
QCS = 32.0             # qc activation fp8 scale
QS = 16.0              # q fp8 scale inside Qpad
NBF16 = np.dtype(ml_dtypes.bfloat16)
NFP8 = np.dtype(ml_dtypes.float8_e4m3)
SEQ = [0, 7, 1, 6, 2, 5, 3, 4]   # packed slot order

_CACHE: dict = {}


def _emit(nc: bass.Bass, tc: tile.TileContext, ctx: ExitStack, io: dict,
          cstar: tuple):
    # ---- packed geometry (compile-time) ----
    seqc = [cstar[s] for s in SEQ]
    offs = np.concatenate([[0], np.cumsum(seqc)])
    NCH = int(offs[-1])
    W = NCH * 128
    owner = []                       # chunk idx -> slot
    for p, s in enumerate(SEQ):
        owner += [s] * seqc[p]
    NW = (NCH + 3) // 4              # 512-col score windows
    NS = (NW + 1) // 2               # keys DMA slabs (2 windows each)

    pool = ctx.enter_context(tc.tile_pool(name="main", bufs=1))
    kpool = ctx.enter_context(tc.tile_pool(name="keys", bufs=2 * NS))
    ebpool = ctx.enter_context(tc.tile_pool(name="eb", bufs=2))
    wpool = ctx.enter_context(tc.tile_pool(name="wstream", bufs=2))
    evpool = ctx.enter_context(tc.tile_pool(name="evt", bufs=6))
    psum = ctx.enter_context(tc.tile_pool(name="ps", bufs=2, space="PSUM"))
    spsum = ctx.enter_context(tc.tile_pool(name="ps2", bufs=2, space="PSUM"))
    rpsum = ctx.enter_context(tc.tile_pool(name="ps3", bufs=1, space="PSUM"))
    scps = ctx.enter_context(tc.tile_pool(name="ps4", bufs=2, space="PSUM"))

    identb = pool.tile([128, 128], BF16)
    make_identity(nc, identb[:])

    def bias_tile(name, nch, eng=None):
        t = pool.tile([128, nch], F32, tag="b" + name)
        (eng or nc.sync).dma_start(t[:], io[name][:])
        return t

    # ------------- per-window validity masks (early, vector is idle) --------
    offW = pool.tile([B, NW], F32)
    nc.sync.dma_start(offW[:], io["offW"][:])
    endW = pool.tile([B, NW], F32)
    nc.sync.dma_start(endW[:], io["endW"][:])
    iot = pool.tile([B, 512], F32)
    nc.gpsimd.iota(iot[:], pattern=[[1, 512]], base=0, channel_multiplier=0,
                   allow_small_or_imprecise_dtypes=True)
    valids = []
    for w in range(NW):
        m1 = pool.tile([B, 512], BF16, tag=f"m1_{w}")
        nc.vector.tensor_scalar(out=m1[:], in0=iot[:],
                                scalar1=offW[:, w:w + 1], scalar2=None,
                                op0=OP.is_ge)
        v = pool.tile([B, 512], BF16, tag=f"va_{w}")
        nc.vector.tensor_scalar(out=v[:], in0=iot[:],
                                scalar1=endW[:, w:w + 1], scalar2=None,
                                op0=OP.is_lt)
        nc.vector.tensor_tensor(out=v[:], in0=v[:], in1=m1[:], op=OP.mult)
        valids.append(v)

    # ---------------- Phase A: q-side MLP (fp8, DoubleRow Wq) -------------
    stateT_n = pool.tile([128, SDIM // 128, BL], FP8)
    nc.sync.dma_start(stateT_n[:], io["stateT"][:])
    latT_n = pool.tile([128, LAT // 128, BL], BF16)
    nc.sync.dma_start(latT_n[:], io["latT"][:])

    bst = bias_tile("b_state", 2)        # x32
    bcq1 = bias_tile("bcq1", 4)          # x32
    bcq2 = bias_tile("bcq2", 4)          # x32
    bq = bias_tile("bq", 32)             # x(32*QCS)

    stateT = [stateT_n[:, c, :] for c in range(SDIM // 128)]
    latT = [latT_n[:, c, :] for c in range(LAT // 128)]

    def layer_T(xT_chunks, w_name, b_tile, n_out, tag, wdt=BF16, scale=None,
                out_dt=BF16, eng=None):
        nk = len(xT_chunks)
        w = wpool.tile([128, nk, n_out], wdt,
                       tag="Wstg8" if wdt == FP8 else "Wstgb")
        (eng or nc.sync).dma_start(w[:], io[w_name][:])
        outs = []
        for j in range(n_out // 128):
            ps = psum.tile([128, BL], F32, tag="sm")
            for k in range(nk):
                nc.tensor.matmul(ps[:], w[:, k, j * 128:(j + 1) * 128],
                                 xT_chunks[k], start=(k == 0),
                                 stop=(k == nk - 1), skip_group_check=True)
            t = pool.tile([128, BL], out_dt, tag=f"{tag}{j}")
            if scale is None:
                nc.vector.tensor_scalar(out=t[:], in0=ps[:],
                                        scalar1=b_tile[:, j:j + 1],
                                        scalar2=None, op0=OP.add)
            else:
                nc.vector.tensor_scalar(out=t[:], in0=ps[:],
                                        scalar1=b_tile[:, j:j + 1],
                                        scalar2=scale, op0=OP.add,
                                        op1=OP.mult)
            outs.append(t[:])
        return outs

    RW = 1.0 / WSCALE
    xT = layer_T(stateT, "W_state", bst, MEMB, "xT", wdt=FP8, scale=RW) + latT
    h1T = layer_T(xT, "Wcq1", bcq1, HID, "h1", wdt=FP8, scale=RW,
                  eng=nc.scalar)
    # qc layer -> single fp8 tile (x QCS), consumed as DoubleRow lhsT.
    # Padded to QCW columns: dual-fp8 LDWEIGHTS rejects 8-wide loads.
    QCW = 32
    qcT = pool.tile([128, KC, QCW], FP8)
    nc.gpsimd.memset(qcT[:], 0.0)
    wcq2 = wpool.tile([128, KC, KD], FP8, tag="Wstg8")
    nc.sync.dma_start(wcq2[:], io["Wcq2"][:])
    for j in range(KC):
        ps = psum.tile([128, BL], F32, tag="sm")
        for k in range(KC):
            nc.tensor.matmul(ps[:], wcq2[:, k, j * 128:(j + 1) * 128],
                             h1T[k], start=(k == 0), stop=(k == KC - 1),
                             skip_group_check=True)
        nc.vector.tensor_scalar(out=qcT[:, j, 0:BL], in0=ps[:],
                                scalar1=bcq2[:, j:j + 1], scalar2=QCS / 32.0,
                                op0=OP.add, op1=OP.mult)

    # Wq in DoubleRow fp8: out [8, 512] per (jg, kcp), then transpose and
    # scatter into Qpad (fp8, xQS) diagonal windows.
    Qpad = pool.tile([128, 2, 2, BL, 72], FP8)
    nc.gpsimd.memset(Qpad[:], 0.0)
    wq = pool.tile([128, 2, 2, H * KD], FP8)
    for kcp in range(2):
        (nc.sync if kcp == 0 else nc.scalar).dma_start(
            wq[:, kcp, :, :], io["Wq"][:, kcp, :, :])
    QSC = QS / (32.0 * QCS)
    for jg in range(8):
        ps = spsum.tile([QCW, 512], F32, tag="sp")
        for kcp in range(2):
            nc.tensor.matmul(ps[:], qcT[:, 2 * kcp:2 * kcp + 2, :],
                             wq[:, kcp, :, jg * 512:(jg + 1) * 512],
                             start=(kcp == 0), stop=(kcp == 1),
                             perf_mode=DR, skip_group_check=True)
        qsb = pool.tile([BL, 512], BF16, tag="qsb")
        nc.scalar.copy(qsb[:], ps[0:BL, :])
        for jj in range(4):
            j = jg * 4 + jj
            h, kc = j // KC, j % KC
            tp = psum.tile([128, BL], BF16, tag="sm")
            nc.tensor.transpose(tp[:], qsb[:, jj * 128:(jj + 1) * 128],
                                identb[0:BL, 0:BL])
            nc.vector.tensor_scalar(
                out=Qpad[:, kc // 2, kc % 2, :, h], in0=tp[:],
                scalar1=bq[:, j:j + 1], scalar2=QSC, op0=OP.add, op1=OP.mult)
    qwin = [Qpad[:, kcp, :, :, :].rearrange("p i b c -> p i (b c)")
            for kcp in range(2)]

    # ---------------- keys slabs (fp8, DoubleRow layout) --------------------
    slabs = []          # (tile, chunk0, nchunks)
    for si in range(NS):
        c0, c1 = 8 * si, min(8 * si + 8, NCH)
        kts = []
        for kcp in range(2):
            kt = kpool.tile([128, 2, 1024], FP8, tag="kt")
            (nc.sync if kcp == 0 else nc.scalar).dma_start(
                kt[:, :, 0:(c1 - c0) * 128],
                io["keysT"][:, kcp, :, c0 * 128:c1 * 128])
            kts.append(kt)
        slabs.append((kts, c0, c1 - c0))

    # ---------------- vals + wagg streams (resident, after keys) ------------
    vengs = [nc.scalar, nc.gpsimd, nc.scalar, nc.gpsimd]
    vres = pool.tile([128, NCH, VD], BF16)
    for p in range(BL):
        vengs[(p // 2) % 4].dma_start(
            vres[:, int(offs[p]):int(offs[p + 1]), :],
            io["vals"][:, int(offs[p]):int(offs[p + 1]), :])
    wagg = pool.tile([128, 32, VD], BF16)
    for gi in range(4):
        (nc.gpsimd if gi % 2 == 0 else nc.sync).dma_start(
            wagg[:, gi * 8:(gi + 1) * 8, :],
            io["Wagg"][:, gi * 8:(gi + 1) * 8, :])

    # ---------------- scores -> exp -> EV -> value matmul, pipelined --------
    EV = pool.tile([B, W], BF16)
    rps = rpsum.tile([B, VD], F32, tag="rp")

    sgs = [None] * NW
    Zg = pool.tile([B, NW], F32)

    def post(w):
        # exp -> mask-mult -> partial Z -> transpose -> value matmuls
        c0, c1 = 4 * w, min(4 * w + 4, NCH)
        gw = (c1 - c0) * 128
        eb = ebpool.tile([B, 512], BF16, tag="eb")
        nc.scalar.activation(eb[:, 0:gw], sgs[w][:, 0:gw], AF.Exp, bias=0.0,
                             scale=1.0 / (KSCALE * QS))
        nc.vector.tensor_tensor(out=EV[:, c0 * 128:c0 * 128 + gw],
                                in0=eb[:, 0:gw], in1=valids[w][:, 0:gw],
                                op=OP.mult)
        nc.vector.tensor_reduce(out=Zg[:, w:w + 1],
                                in_=EV[:, c0 * 128:c0 * 128 + gw],
                                op=OP.add, axis=mybir.AxisListType.X)
        for i in range(c0, c1):
            tpp = psum.tile([128, B], BF16, tag="sm")
            nc.tensor.transpose(tpp[:], EV[:, i * 128:(i + 1) * 128],
                                identb[0:B, 0:B])
            evt = evpool.tile([128, B], BF16, tag="evt")
            nc.scalar.copy(evt[:], tpp[:])
            nc.tensor.matmul(rps[:], evt[:], vres[:, i, :],
                             start=(i == 0), stop=(i == NCH - 1),
                             skip_group_check=True)

    for w in range(NW):
        c0, c1 = 4 * w, min(4 * w + 4, NCH)
        sg = scps.tile([B, 512], F32, tag="sg")
        sgs[w] = sg
        # matmul pieces: runs of chunks with the same owner slot
        i = c0
        while i < c1:
            j = i
            while j < c1 and owner[j] == owner[i]:
                j += 1
            s = owner[i]
            si, sc0 = i // 8, (i % 8) * 128
            lo, cw = (i - c0) * 128, (j - i) * 128
            kts = slabs[si][0]
            for kcp in range(2):
                nc.tensor.matmul(
                    sg[:, lo:lo + cw],
                    qwin[kcp][:, :, s * 64:s * 64 + 64],
                    kts[kcp][:, :, sc0:sc0 + cw],
                    start=(kcp == 0), stop=(kcp == 1),
                    perf_mode=DR, skip_group_check=True)
            i = j
        if w > 0:
            post(w - 1)
    post(NW - 1)

    # Z over the whole masked-prob image; R = 1/Z folded into the readout
    Zh = pool.tile([B, 1], F32)
    nc.vector.tensor_reduce(out=Zh[:], in_=Zg[:], op=OP.add,
                            axis=mybir.AxisListType.X)
    R = pool.tile([B, 1], F32)
    nc.vector.reciprocal(R[:], Zh[:])
    rsb = pool.tile([B, VD], BF16, tag="rs")
    nc.vector.tensor_scalar(out=rsb[:], in0=rps[:], scalar1=R[:, 0:1],
                            scalar2=None, op0=OP.mult)
    RT = pool.tile([128, VD // 128, B], BF16)
    for vc in range(VD // 128):
        tr = psum.tile([128, B], BF16, tag="sm")
        nc.tensor.transpose(tr[:], rsb[:, vc * 128:(vc + 1) * 128],
                            identb[0:B, 0:B])
        nc.vector.tensor_copy(RT[:, vc, :], tr[:])

    # ---------------- Phase E: output MLP chain (bf16) ---------------------
    bagg = bias_tile("bagg", 4)
    brk1 = bias_tile("brk1", 4)
    brv1 = bias_tile("brv1", 4, eng=nc.scalar)

    aggp = spsum.tile([BL, VD], F32, tag="sp")
    for c in range(32):
        h, vc = c // 4, c % 4
        nc.tensor.matmul(aggp[:], RT[:, vc, h:B:H], wagg[:, c, :],
                         start=(c == 0), stop=(c == 31),
                         skip_group_check=True)
    aggsb = pool.tile([BL, VD], BF16, tag="aggsb")
    nc.scalar.copy(aggsb[:], aggp[:])
    AT = []
    for j in range(VD // 128):
        tp = psum.tile([128, BL], BF16, tag="sm")
        nc.tensor.transpose(tp[:], aggsb[:, j * 128:(j + 1) * 128],
                            identb[0:BL, 0:BL])
        t = pool.tile([128, BL], BF16, tag=f"AT{j}")
        nc.vector.tensor_scalar(out=t[:], in0=tp[:],
                                scalar1=bagg[:, j:j + 1],
                                scalar2=None, op0=OP.add)
        AT.append(t[:])

    ones = pool.tile([1, BL], F32)
    nc.gpsimd.memset(ones[:], 1.0)

    def bias_bcast(name, eng=None):
        brow = pool.tile([1, 512], F32, tag="br" + name)
        (eng or nc.sync).dma_start(brow[:], io[name][:])
        bb = spsum.tile([BL, 512], F32, tag="sp")
        nc.tensor.matmul(bb[:], ones[:], brow[:], start=True, stop=True)
        bsb = pool.tile([BL, 512], F32, tag="bs" + name)
        nc.vector.tensor_copy(bsb[:], bb[:])
        return bsb

    bk2 = bias_bcast("brk2_flat")
    bv2 = bias_bcast("brv2_flat", eng=nc.scalar)

    def layer_nat(xT_chunks, w_name, n_out, eng=None):
        nk = len(xT_chunks)
        w = wpool.tile([128, nk, n_out], BF16, tag="Wstgb")
        (eng or nc.sync).dma_start(w[:], io[w_name][:])
        ps = spsum.tile([BL, n_out], F32, tag="sp")
        for k in range(nk):
            nc.tensor.matmul(ps[:], xT_chunks[k], w[:, k, :],
                             start=(k == 0), stop=(k == nk - 1),
                             skip_group_check=True)
        return ps

    hkT = layer_T(AT, "Wrk1", brk1, HID, "hk")
    ok_ps = layer_nat(hkT, "Wrk2", RIMQ)
    hvT = layer_T(AT, "Wrv1", brv1, HID, "hv", eng=nc.scalar)
    ov_ps = layer_nat(hvT, "Wrv2", VD, eng=nc.scalar)

    for name, ps_, bias_sb in (("out_key", ok_ps, bk2), ("out_val", ov_ps, bv2)):
        onat = pool.tile([BL, 512], F32, tag="o" + name)
        nc.vector.tensor_tensor(out=onat[:], in0=ps_[:], in1=bias_sb[:],
                                op=OP.add)
        nc.sync.dma_start(io[name][:], onat[:])


def _build(cstar):
    seqc = [cstar[s] for s in SEQ]
    NCH = int(sum(seqc))
    W = NCH * 128
    NW = (NCH + 3) // 4
    nc = bacc.Bacc("TRN2", target_bir_lowering=False, debug=False,
                   num_devices=NCORES)
    io = {}

    def din(name, shape, dt=BF16):
        io[name] = nc.dram_tensor(name, shape, dt, kind="ExternalInput").ap()

    din("keysT", [128, 2, 2, W], FP8)
    din("vals", [128, NCH, VD])
    din("offW", [B, NW], F32)
    din("endW", [B, NW], F32)
    din("stateT", [128, SDIM // 128, BL], FP8)
    din("latT", [128, LAT // 128, BL])
    din("W_state", [128, KC, MEMB], FP8)
    din("b_state", [128, 2], F32)
    din("Wcq1", [128, KC, HID], FP8)
    din("bcq1", [128, 4], F32)
    din("Wcq2", [128, KC, KD], FP8)
    din("bcq2", [128, 4], F32)
    din("Wq", [128, 2, 2, H * KD], FP8)
    din("bq", [128, 32], F32)
    din("Wagg", [128, 32, VD])
    din("bagg", [128, 4], F32)
    din("Wrk1", [128, KC, HID])
    din("brk1", [128, 4], F32)
    din("Wrk2", [128, KC, RIMQ])
    din("brk2_flat", [1, 512], F32)
    din("Wrv1", [128, KC, HID])
    din("brv1", [128, 4], F32)
    din("Wrv2", [128, KC, VD])
    din("brv2_flat", [1, 512], F32)
    io["out_key"] = nc.dram_tensor("out_key", [BL, RIMQ], F32,
                                   kind="ExternalOutput").ap()
    io["out_val"] = nc.dram_tensor("out_val", [BL, VD], F32,
                                   kind="ExternalOutput").ap()

    with tile.TileContext(nc) as tc, ExitStack() as ctx:
        _emit(nc, tc, ctx, io, cstar)
    nc.compile()
    return nc


def _rsb(bias, nch, scale=1.0):
    return np.ascontiguousarray(
        np.asarray(bias, np.float32).reshape(nch, 128).T * scale)


def _wchunk(w, dt=NBF16, scale=1.0):
    w = np.asarray(w, np.float32) * scale
    f, c = w.shape
    return np.ascontiguousarray(
        w.reshape(f // 128, 128, c).transpose(1, 0, 2)).astype(dt)


def _actT(x, dt):
    x = np.asarray(x, np.float32)
    bl, f = x.shape
    return np.ascontiguousarray(
        x.T.reshape(f // 128, 128, bl).transpose(1, 0, 2)).astype(dt)


def _plan(step):
    cb = np.clip((np.asarray(step, np.int64) + 127) // 128, 1, 8)
    order = np.argsort(-cb, kind="stable")
    cstar = tuple(int(cb[order[8 * s]]) for s in range(BL))
    return order, cstar


def _shard(inputs):
    f = lambda x: np.asarray(x, np.float32)
    keys, vals, rpe = f(inputs["keys"]), f(inputs["vals"]), f(inputs["rpe_mod"])
    step = np.asarray(inputs["step"]).astype(np.int64)
    state, lat = f(inputs["state"]), f(inputs["task_inference_latent"])

    order, cstar = _plan(step)
    seqc = [cstar[s] for s in SEQ]
    offs = np.concatenate([[0], np.cumsum(seqc)])
    NCH = int(offs[-1])
    NW = (NCH + 3) // 4

    shared = {
        "W_state": _wchunk(inputs["W_state"], NFP8, WSCALE),
        "b_state": _rsb(inputs["b_state"], 2, WSCALE),
        "Wcq1": _wchunk(inputs["Wcq1"], NFP8, WSCALE),
        "bcq1": _rsb(inputs["bcq1"], 4, WSCALE),
        "Wcq2": _wchunk(inputs["Wcq2"], NFP8, WSCALE),
        "bcq2": _rsb(inputs["bcq2"], 4, WSCALE),
        "Wq": _wchunk(inputs["Wq"], NFP8, WSCALE).reshape(128, 2, 2, H * KD),
        "bq": _rsb(inputs["bq"], 32, WSCALE * QCS),
        "Wagg": _wchunk(inputs["Wagg"]),
        "bagg": _rsb(inputs["bagg"], 4),
        "Wrk1": _wchunk(inputs["Wrk1"]), "brk1": _rsb(inputs["brk1"], 4),
        "Wrk2": _wchunk(inputs["Wrk2"]),
        "brk2_flat": np.ascontiguousarray(f(inputs["brk2"])[None, :]),
        "Wrv1": _wchunk(inputs["Wrv1"]), "brv1": _rsb(inputs["brv1"], 4),
        "Wrv2": _wchunk(inputs["Wrv2"]),
        "brv2_flat": np.ascontiguousarray(f(inputs["brv2"])[None, :]),
    }
    kfold = keys * rpe * (KSCALE * RSQK)            # [L, 64, K]
    in_maps = []
    for m in range(NCORES):
        envs = [int(order[8 * s + m]) for s in range(BL)]
        kp = np.zeros((128, 2, 2, NCH * 128), NFP8)
        vp = np.zeros((128, NCH, VD), NBF16)
        offW = np.zeros((B, NW), np.float32)
        endW = np.zeros((B, NW), np.float32)
        for p, s in enumerate(SEQ):
            e = envs[s]
            nl = cstar[s] * 128
            c0, c1 = int(offs[p]), int(offs[p + 1])
            kb = kfold[:nl, e, :].T.reshape(2, 2, 128, nl).transpose(
                2, 0, 1, 3)
            kp[:, :, :, c0 * 128:c1 * 128] = kb.astype(NFP8)
            vb = vals[:nl, e, :].reshape(cstar[s], 128, VD).transpose(1, 0, 2)
            vp[:, c0:c1, :] = vb.astype(NBF16)
            for w in range(NW):
                offW[s * H:(s + 1) * H, w] = c0 * 128 - w * 512
                endW[s * H:(s + 1) * H, w] = (c0 * 128 - w * 512
                                              + float(step[e]))
        in_maps.append({
            "keysT": kp, "vals": vp, "offW": offW, "endW": endW,
            "stateT": _actT(state[envs], NFP8),
            "latT": _actT(lat[envs], NBF16),
            **shared,
        })
    return in_maps, order


def kernel(**inputs):
    order, cstar = _plan(inputs["step"])
    nc = _CACHE.get(cstar)
    if nc is None:
        nc = _CACHE[cstar] = _build(cstar)
    in_maps, order = _shard(inputs)
    res = run_bass_kernel_spmd(nc, in_maps, list(range(NCORES)),
                               **_CACHE.get("run_kwargs", {}))
    _CACHE["last_result"] = res
    ok = np.empty((B, RIMQ), np.float32)
    ov = np.empty((B, VD), np.float32)
    for m in range(NCORES):
        for s in range(BL):
            e = int(order[8 * s + m])
            ok[e] = res.results[m]["out_key"][s]
            ov[e] = res.results[m]["out_val"][s]
    return ok[:, None, :], ov[:, None, :]


# revision 31
# speedup vs baseline: 1.2263x; 1.0124x over previous
"""DND retrieval (episodic memory read) kernel for 8 Trainium2 NeuronCores.

Data-parallel over batch B=64 -> 8 envs per core, with step-aware
packing: only ceil(step/128) l-chunks per env are ever touched (the
rest are masked to zero by the softmax validity mask), so the host
packs exactly those chunks, assigns envs to cores by sorted rank so
every core shares one compiled chunk pattern C*, and the kernel skips
the dead ~45% of keys/vals DMA and PE work.

Precision: keys (with rpe * 64/sqrt(K) folded in) and the q-side MLP
stream as fp8e4m3 (weights x32, qc x32, q x16 host/chip scales); the
scores and Wq matmuls run in fp8 DoubleRow mode (2 contraction rows
per partition, 2x PE rate). vals and output-side weights stay bf16
(fp8 there pushes error past budget).

Scores are processed in 512-column windows of the packed image through
a 2-bank PSUM ring: scores -> exp(S/1024) -> multiply by a precomputed
validity mask -> unnormalized probs transpose straight into the value
matmul; softmax 1/Z is applied to the [64, 512] result instead
(linearity), so nothing waits on the global sum. Scores are tiny
(|s| < 0.3), so no max pass is needed.
"""
from contextlib import ExitStack

import numpy as np
import ml_dtypes

import concourse.bass as bass
import concourse.tile as tile
from concourse import bacc, mybir
from concourse.bass_utils import run_bass_kernel_spmd
from concourse.masks import make_identity

F32 = mybir.dt.float32
BF16 = mybir.dt.bfloat16
FP8 = mybir.dt.float8e4
AF = mybir.ActivationFunctionType
OP = mybir.AluOpType
DR = mybir.MatmulPerfMode.DoubleRow

L = 1024
B = 64        # rows of the batched softmax image: (slot, head)
BL = 8        # envs (slots) per core
KD = 512
VD = 512
H = 8
MEMB = 256
SDIM = 512
HID = 512
RIMQ = 512
LAT = KD - MEMB
NCORES = 8
KC = KD // 128
RSQK = 1.0 / np.sqrt(np.float32(KD))
KSCALE = 64.0          # folded into keys on host
WSCALE = 32.0          # fp8 weight scale
QCS = 32.0             # qc activation fp8 scale
QS = 16.0              # q fp8 scale inside Qpad
NBF16 = np.dtype(ml_dtypes.bfloat16)
NFP8 = np.dtype(ml_dtypes.float8_e4m3)
SEQ = [0, 7, 1, 6, 2, 5, 3, 4]   # packed slot order

_CACHE: dict = {}


def _emit(nc: bass.Bass, tc: tile.TileContext, ctx: ExitStack, io: dict,
          cstar: tuple):
    # ---- packed geometry (compile-time) ----
    seqc = [cstar[s] for s in SEQ]
    offs = np.concatenate([[0], np.cumsum(seqc)])
    NCH = int(offs[-1])
    W = NCH * 128
    owner = []                       # chunk idx -> slot
    for p, s in enumerate(SEQ):
        owner += [s] * seqc[p]
    NW = (NCH + 3) // 4              # 512-col score windows
    NS = (NW + 1) // 2               # keys DMA slabs (2 windows each)

    pool = ctx.enter_context(tc.tile_pool(name="main", bufs=1))
    kpool = ctx.enter_context(tc.tile_pool(name="keys", bufs=2 * NS))
    ebpool = ctx.enter_context(tc.tile_pool(name="eb", bufs=2))
    wpool = ctx.enter_context(tc.tile_pool(name="wstream", bufs=2))
    evpool = ctx.enter_context(tc.tile_pool(name="evt", bufs=6))
    psum = ctx.enter_context(tc.tile_pool(name="ps", bufs=2, space="PSUM"))
    spsum = ctx.enter_context(tc.tile_pool(name="ps2", bufs=2, space="PSUM"))
    rpsum = ctx.enter_context(tc.tile_pool(name="ps3", bufs=1, space="PSUM"))
    scps = ctx.enter_context(tc.tile_pool(name="ps4", bufs=2, space="PSUM"))

    identb = pool.tile([128, 128], BF16)
    make_identity(nc, identb[:])

    def bias_tile(name, nch, eng=None):
        t = pool.tile([128, nch], F32, tag="b" + name)
        (eng or nc.sync).dma_start(t[:], io[name][:])
        return t

    # ------------- per-window validity masks (early, vector is idle) --------
    offW = pool.tile([B, NW], F32)
    nc.sync.dma_start(offW[:], io["offW"][:])
    endW = pool.tile([B, NW], F32)
    nc.sync.dma_start(endW[:], io["endW"][:])
    iot = pool.tile([B, 512], F32)
    nc.gpsimd.iota(iot[:], pattern=[[1, 512]], base=0, channel_multiplier=0,
                   allow_small_or_imprecise_dtypes=True)
    valids = []
    for w in range(NW):
        m1 = pool.tile([B, 512], BF16, tag=f"m1_{w}")
        nc.vector.tensor_scalar(out=m1[:], in0=iot[:],
                                scalar1=offW[:, w:w + 1], scalar2=None,
                                op0=OP.is_ge)
        v = pool.tile([B, 512], BF16, tag=f"va_{w}")
        nc.vector.tensor_scalar(out=v[:], in0=iot[:],
                                scalar1=endW[:, w:w + 1], scalar2=None,
                                op0=OP.is_lt)
        nc.vector.tensor_tensor(out=v[:], in0=v[:], in1=m1[:], op=OP.mult)
        valids.append(v)

    # ---------------- Phase A: q-side MLP (fp8, DoubleRow Wq) -------------
    stateT_n = pool.tile([128, SDIM // 128, BL], FP8)
    nc.sync.dma_start(stateT_n[:], io["stateT"][:])
    latT_n = pool.tile([128, LAT // 128, BL], BF16)
    nc.sync.dma_start(latT_n[:], io["latT"][:])

    bst = bias_tile("b_state", 2)        # x32
    bcq1 = bias_tile("bcq1", 4)          # x32
    bcq2 = bias_tile("bcq2", 4)          # x32
    bq = bias_tile("bq", 32)             # x(32*QCS)

    stateT = [stateT_n[:, c, :] for c in range(SDIM // 128)]
    latT = [latT_n[:, c, :] for c in range(LAT // 128)]

    def layer_T(xT_chunks, w_name, b_tile, n_out, tag, wdt=BF16, scale=None,
                out_dt=BF16, eng=None):
        nk = len(xT_chunks)
        w = wpool.tile([128, nk, n_out], wdt,
                       tag="Wstg8" if wdt == FP8 else "Wstgb")
        (eng or nc.sync).dma_start(w[:], io[w_name][:])
        outs = []
        for j in range(n_out // 128):
            ps = psum.tile([128, BL], F32, tag="sm")
            for k in range(nk):
                nc.tensor.matmul(ps[:], w[:, k, j * 128:(j + 1) * 128],
                                 xT_chunks[k], start=(k == 0),
                                 stop=(k == nk - 1), skip_group_check=True)
            t = pool.tile([128, BL], out_dt, tag=f"{tag}{j}")
            if scale is None:
                nc.vector.tensor_scalar(out=t[:], in0=ps[:],
                                        scalar1=b_tile[:, j:j + 1],
                                        scalar2=None, op0=OP.add)
            else:
                nc.vector.tensor_scalar(out=t[:], in0=ps[:],
                                        scalar1=b_tile[:, j:j + 1],
                                        scalar2=scale, op0=OP.add,
                                        op1=OP.mult)
            outs.append(t[:])
        return outs

    RW = 1.0 / WSCALE
    xT = layer_T(stateT, "W_state", bst, MEMB, "xT", wdt=FP8, scale=RW) + latT
    h1T = layer_T(xT, "Wcq1", bcq1, HID, "h1", wdt=FP8, scale=RW,
                  eng=nc.scalar)
    # qc layer -> single fp8 tile (x QCS), consumed as DoubleRow lhsT.
    # Padded to QCW columns: dual-fp8 LDWEIGHTS rejects 8-wide loads.
    QCW = 32
    qcT = pool.tile([128, KC, QCW], FP8)
    nc.gpsimd.memset(qcT[:], 0.0)
    wcq2 = wpool.tile([128, KC, KD], FP8, tag="Wstg8")
    nc.sync.dma_start(wcq2[:], io["Wcq2"][:])
    for j in range(KC):
        ps = psum.tile([128, BL], F32, tag="sm")
        for k in range(KC):
            nc.tensor.matmul(ps[:], wcq2[:, k, j * 128:(j + 1) * 128],
                             h1T[k], start=(k == 0), stop=(k == KC - 1),
                             skip_group_check=True)
        nc.vector.tensor_scalar(out=qcT[:, j, 0:BL], in0=ps[:],
                                scalar1=bcq2[:, j:j + 1], scalar2=QCS / 32.0,
                                op0=OP.add, op1=OP.mult)

    # Wq in DoubleRow fp8: out [8, 512] per (jg, kcp), then transpose and
    # scatter into Qpad (fp8, xQS) diagonal windows.
    Qpad = pool.tile([128, 2, 2, BL, 72], FP8)
    nc.gpsimd.memset(Qpad[:], 0.0)
    wq = pool.tile([128, 2, 2, H * KD], FP8)
    for kcp in range(2):
        (nc.sync if kcp == 0 else nc.scalar).dma_start(
            wq[:, kcp, :, :], io["Wq"][:, kcp, :, :])
    QSC = QS / (32.0 * QCS)
    for jg in range(8):
        ps = spsum.tile([QCW, 512], F32, tag="sp")
        for kcp in range(2):
            nc.tensor.matmul(ps[:], qcT[:, 2 * kcp:2 * kcp + 2, :],
                             wq[:, kcp, :, jg * 512:(jg + 1) * 512],
                             start=(kcp == 0), stop=(kcp == 1),
                             perf_mode=DR, skip_group_check=True)
        qsb = pool.tile([BL, 512], BF16, tag="qsb")
        nc.scalar.copy(qsb[:], ps[0:BL, :])
        for jj in range(4):
            j = jg * 4 + jj
            h, kc = j // KC, j % KC
            tp = psum.tile([128, BL], BF16, tag="sm")
            nc.tensor.transpose(tp[:], qsb[:, jj * 128:(jj + 1) * 128],
                                identb[0:BL, 0:BL])
            nc.vector.tensor_scalar(
                out=Qpad[:, kc // 2, kc % 2, :, h], in0=tp[:],
                scalar1=bq[:, j:j + 1], scalar2=QSC, op0=OP.add, op1=OP.mult)
    qwin = [Qpad[:, kcp, :, :, :].rearrange("p i b c -> p i (b c)")
            for kcp in range(2)]

    # ---------------- keys slabs (fp8, DoubleRow layout) --------------------
    slabs = []          # (tile, chunk0, nchunks)
    for si in range(NS):
        c0, c1 = 8 * si, min(8 * si + 8, NCH)
        kts = []
        for kcp in range(2):
            kt = kpool.tile([128, 2, 1024], FP8, tag="kt")
            (nc.sync if kcp == 0 else nc.scalar).dma_start(
                kt[:, :, 0:(c1 - c0) * 128],
                io["keysT"][:, kcp, :, c0 * 128:c1 * 128])
            kts.append(kt)
        slabs.append((kts, c0, c1 - c0))

    # ---------------- vals + wagg streams (resident, after keys) ------------
    vengs = [nc.scalar, nc.gpsimd, nc.scalar, nc.gpsimd]
    vres = pool.tile([128, NCH, VD], BF16)
    for p in range(BL):
        vengs[(p // 2) % 4].dma_start(
            vres[:, int(offs[p]):int(offs[p + 1]), :],
            io["vals"][:, int(offs[p]):int(offs[p + 1]), :])
    wagg = pool.tile([128, 32, VD], BF16)
    for gi in range(4):
        (nc.gpsimd if gi % 2 == 0 else nc.scalar).dma_start(
            wagg[:, gi * 8:(gi + 1) * 8, :],
            io["Wagg"][:, gi * 8:(gi + 1) * 8, :])

    # ---------------- scores -> exp -> EV -> value matmul, pipelined --------
    EV = pool.tile([B, W], BF16)
    rps = rpsum.tile([B, VD], F32, tag="rp")

    sgs = [None] * NW
    Zg = pool.tile([B, NW], F32)

    def post(w):
        # exp -> mask-mult -> partial Z -> transpose -> value matmuls
        c0, c1 = 4 * w, min(4 * w + 4, NCH)
        gw = (c1 - c0) * 128
        eb = ebpool.tile([B, 512], BF16, tag="eb")
        nc.scalar.activation(eb[:, 0:gw], sgs[w][:, 0:gw], AF.Exp, bias=0.0,
                             scale=1.0 / (KSCALE * QS))
        nc.vector.tensor_tensor(out=EV[:, c0 * 128:c0 * 128 + gw],
                                in0=eb[:, 0:gw], in1=valids[w][:, 0:gw],
                                op=OP.mult)
        nc.vector.tensor_reduce(out=Zg[:, w:w + 1],
                                in_=EV[:, c0 * 128:c0 * 128 + gw],
                                op=OP.add, axis=mybir.AxisListType.X)
        for i in range(c0, c1):
            tpp = psum.tile([128, B], BF16, tag="sm")
            nc.tensor.transpose(tpp[:], EV[:, i * 128:(i + 1) * 128],
                                identb[0:B, 0:B])
            evt = evpool.tile([128, B], BF16, tag="evt")
            nc.scalar.copy(evt[:], tpp[:])
            nc.tensor.matmul(rps[:], evt[:], vres[:, i, :],
                             start=(i == 0), stop=(i == NCH - 1),
                             skip_group_check=True)

    for w in range(NW):
        c0, c1 = 4 * w, min(4 * w + 4, NCH)
        sg = scps.tile([B, 512], F32, tag="sg")
        sgs[w] = sg
        # matmul pieces: runs of chunks with the same owner slot
        i = c0
        while i < c1:
            j = i
            while j < c1 and owner[j] == owner[i]:
                j += 1
            s = owner[i]
            si, sc0 = i // 8, (i % 8) * 128
            lo, cw = (i - c0) * 128, (j - i) * 128
            kts = slabs[si][0]
            for kcp in range(2):
                nc.tensor.matmul(
                    sg[:, lo:lo + cw],
                    qwin[kcp][:, :, s * 64:s * 64 + 64],
                    kts[kcp][:, :, sc0:sc0 + cw],
                    start=(kcp == 0), stop=(kcp == 1),
                    perf_mode=DR, skip_group_check=True)
            i = j
        if w > 0:
            post(w - 1)
    post(NW - 1)

    # Z over the whole masked-prob image; R = 1/Z folded into the readout
    Zh = pool.tile([B, 1], F32)
    nc.vector.tensor_reduce(out=Zh[:], in_=Zg[:], op=OP.add,
                            axis=mybir.AxisListType.X)
    R = pool.tile([B, 1], F32)
    nc.vector.reciprocal(R[:], Zh[:])
    rsb = pool.tile([B, VD], BF16, tag="rs")
    nc.vector.tensor_scalar(out=rsb[:], in0=rps[:], scalar1=R[:, 0:1],
                            scalar2=None, op0=OP.mult)
    RT = pool.tile([128, VD // 128, B], BF16)
    for vc in range(VD // 128):
        tr = psum.tile([128, B], BF16, tag="sm")
        nc.tensor.transpose(tr[:], rsb[:, vc * 128:(vc + 1) * 128],
                            identb[0:B, 0:B])
        nc.vector.tensor_copy(RT[:, vc, :], tr[:])

    # ---------------- Phase E: output MLP chain (bf16) ---------------------
    bagg = bias_tile("bagg", 4)
    brk1 = bias_tile("brk1", 4)
    brv1 = bias_tile("brv1", 4, eng=nc.scalar)

    aggp = spsum.tile([BL, VD], F32, tag="sp")
    for c in range(32):
        h, vc = c // 4, c % 4
        nc.tensor.matmul(aggp[:], RT[:, vc, h:B:H], wagg[:, c, :],
                         start=(c == 0), stop=(c == 31),
                         skip_group_check=True)
    aggsb = pool.tile([BL, VD], BF16, tag="aggsb")
    nc.scalar.copy(aggsb[:], aggp[:])
    AT = []
    for j in range(VD // 128):
        tp = psum.tile([128, BL], BF16, tag="sm")
        nc.tensor.transpose(tp[:], aggsb[:, j * 128:(j + 1) * 128],
                            identb[0:BL, 0:BL])
        t = pool.tile([128, BL], BF16, tag=f"AT{j}")
        nc.vector.tensor_scalar(out=t[:], in0=tp[:],
                                scalar1=bagg[:, j:j + 1],
                                scalar2=None, op0=OP.add)
        AT.append(t[:])

    ones = pool.tile([1, BL], F32)
    nc.gpsimd.memset(ones[:], 1.0)

    def bias_bcast(name, eng=None):
        brow = pool.tile([1, 512], F32, tag="br" + name)
        (eng or nc.sync).dma_start(brow[:], io[name][:])
        bb = spsum.tile([BL, 512], F32, tag="sp")
        nc.tensor.matmul(bb[:], ones[:], brow[:], start=True, stop=True)
        bsb = pool.tile([BL, 512], F32, tag="bs" + name)
        nc.vector.tensor_copy(bsb[:], bb[:])
        return bsb

    bk2 = bias_bcast("brk2_flat")
    bv2 = bias_bcast("brv2_flat", eng=nc.scalar)

    def layer_nat(xT_chunks, w_name, n_out, eng=None):
        nk = len(xT_chunks)
        w = wpool.tile([128, nk, n_out], BF16, tag="Wstgb")
        (eng or nc.sync).dma_start(w[:], io[w_name][:])
        ps = spsum.tile([BL, n_out], F32, tag="sp")
        for k in range(nk):
            nc.tensor.matmul(ps[:], xT_chunks[k], w[:, k, :],
                             start=(k == 0), stop=(k == nk - 1),
                             skip_group_check=True)
        return ps

    hkT = layer_T(AT, "Wrk1", brk1, HID, "hk")
    ok_ps = layer_nat(hkT, "Wrk2", RIMQ)
    hvT = layer_T(AT, "Wrv1", brv1, HID, "hv", eng=nc.scalar)
    ov_ps = layer_nat(hvT, "Wrv2", VD, eng=nc.scalar)

    for name, ps_, bias_sb in (("out_key", ok_ps, bk2), ("out_val", ov_ps, bv2)):
        onat = pool.tile([BL, 512], F32, tag="o" + name)
        nc.vector.tensor_tensor(out=onat[:], in0=ps_[:], in1=bias_sb[:],
                                op=OP.add)
        nc.sync.dma_start(io[name][:], onat[:])


def _build(cstar):
    seqc = [cstar[s] for s in SEQ]
    NCH = int(sum(seqc))
    W = NCH * 128
    NW = (NCH + 3) // 4
    nc = bacc.Bacc("TRN2", target_bir_lowering=False, debug=False,
                   num_devices=NCORES)
    io = {}

    def din(name, shape, dt=BF16):
        io[name] = nc.dram_tensor(name, shape, dt, kind="ExternalInput").ap()

    din("keysT", [128, 2, 2, W], FP8)
    din("vals", [128, NCH, VD])
    din("offW", [B, NW], F32)
    din("endW", [B, NW], F32)
    din("stateT", [128, SDIM // 128, BL], FP8)
    din("latT", [128, LAT // 128, BL])
    din("W_state", [128, KC, MEMB], FP8)
    din("b_state", [128, 2], F32)
    din("Wcq1", [128, KC, HID], FP8)
    din("bcq1", [128, 4], F32)
    din("Wcq2", [128, KC, KD], FP8)
    din("bcq2", [128, 4], F32)
    din("Wq", [128, 2, 2, H * KD], FP8)
    din("bq", [128, 32], F32)
    din("Wagg", [128, 32, VD])
    din("bagg", [128, 4], F32)
    din("Wrk1", [128, KC, HID])
    din("brk1", [128, 4], F32)
    din("Wrk2", [128, KC, RIMQ])
    din("brk2_flat", [1, 512], F32)
    din("Wrv1", [128, KC, HID])
    din("brv1", [128, 4], F32)
    din("Wrv2", [128, KC, VD])
    din("brv2_flat", [1, 512], F32)
    io["out_key"] = nc.dram_tensor("out_key", [BL, RIMQ], F32,
                                   kind="ExternalOutput").ap()
    io["out_val"] = nc.dram_tensor("out_val", [BL, VD], F32,
                                   kind="ExternalOutput").ap()

    with tile.TileContext(nc) as tc, ExitStack() as ctx:
        _emit(nc, tc, ctx, io, cstar)
    nc.compile()
    return nc


def _rsb(bias, nch, scale=1.0):
    return np.ascontiguousarray(
        np.asarray(bias, np.float32).reshape(nch, 128).T * scale)


def _wchunk(w, dt=NBF16, scale=1.0):
    w = np.asarray(w, np.float32) * scale
    f, c = w.shape
    return np.ascontiguousarray(
        w.reshape(f // 128, 128, c).transpose(1, 0, 2)).astype(dt)


def _actT(x, dt):
    x = np.asarray(x, np.float32)
    bl, f = x.shape
    return np.ascontiguousarray(
        x.T.reshape(f // 128, 128, bl).transpose(1, 0, 2)).astype(dt)


def _plan(step):
    cb = np.clip((np.asarray(step, np.int64) + 127) // 128, 1, 8)
    order = np.argsort(-cb, kind="stable")
    cstar = tuple(int(cb[order[8 * s]]) for s in range(BL))
    return order, cstar


def _shard(inputs):
    f = lambda x: np.asarray(x, np.float32)
    keys, vals, rpe = f(inputs["keys"]), f(inputs["vals"]), f(inputs["rpe_mod"])
    step = np.asarray(inputs["step"]).astype(np.int64)
    state, lat = f(inputs["state"]), f(inputs["task_inference_latent"])

    order, cstar = _plan(step)
    seqc = [cstar[s] for s in SEQ]
    offs = np.concatenate([[0], np.cumsum(seqc)])
    NCH = int(offs[-1])
    NW = (NCH + 3) // 4

    shared = {
        "W_state": _wchunk(inputs["W_state"], NFP8, WSCALE),
        "b_state": _rsb(inputs["b_state"], 2, WSCALE),
        "Wcq1": _wchunk(inputs["Wcq1"], NFP8, WSCALE),
        "bcq1": _rsb(inputs["bcq1"], 4, WSCALE),
        "Wcq2": _wchunk(inputs["Wcq2"], NFP8, WSCALE),
        "bcq2": _rsb(inputs["bcq2"], 4, WSCALE),
        "Wq": _wchunk(inputs["Wq"], NFP8, WSCALE).reshape(128, 2, 2, H * KD),
        "bq": _rsb(inputs["bq"], 32, WSCALE * QCS),
        "Wagg": _wchunk(inputs["Wagg"]),
        "bagg": _rsb(inputs["bagg"], 4),
        "Wrk1": _wchunk(inputs["Wrk1"]), "brk1": _rsb(inputs["brk1"], 4),
        "Wrk2": _wchunk(inputs["Wrk2"]),
        "brk2_flat": np.ascontiguousarray(f(inputs["brk2"])[None, :]),
        "Wrv1": _wchunk(inputs["Wrv1"]), "brv1": _rsb(inputs["brv1"], 4),
        "Wrv2": _wchunk(inputs["Wrv2"]),
        "brv2_flat": np.ascontiguousarray(f(inputs["brv2"])[None, :]),
    }
    kfold = keys * rpe * (KSCALE * RSQK)            # [L, 64, K]
    in_maps = []
    for m in range(NCORES):
        envs = [int(order[8 * s + m]) for s in range(BL)]
        kp = np.zeros((128, 2, 2, NCH * 128), NFP8)
        vp = np.zeros((128, NCH, VD), NBF16)
        offW = np.zeros((B, NW), np.float32)
        endW = np.zeros((B, NW), np.float32)
        for p, s in enumerate(SEQ):
            e = envs[s]
            nl = cstar[s] * 128
            c0, c1 = int(offs[p]), int(offs[p + 1])
            kb = kfold[:nl, e, :].T.reshape(2, 2, 128, nl).transpose(
                2, 0, 1, 3)
            kp[:, :, :, c0 * 128:c1 * 128] = kb.astype(NFP8)
            vb = vals[:nl, e, :].reshape(cstar[s], 128, VD).transpose(1, 0, 2)
            vp[:, c0:c1, :] = vb.astype(NBF16)
            for w in range(NW):
                offW[s * H:(s + 1) * H, w] = c0 * 128 - w * 512
                endW[s * H:(s + 1) * H, w] = (c0 * 128 - w * 512
                                              + float(step[e]))
        in_maps.append({
            "keysT": kp, "vals": vp, "offW": offW, "endW": endW,
            "stateT": _actT(state[envs], NFP8),
            "latT": _actT(lat[envs], NBF16),
            **shared,
        })
    return in_maps, order


def kernel(**inputs):
    order, cstar = _plan(inputs["step"])
    nc = _CACHE.get(cstar)
    if nc is None:
        nc = _CACHE[cstar] = _build(cstar)
    in_maps, order = _shard(inputs)
    res = run_bass_kernel_spmd(nc, in_maps, list(range(NCORES)),
                               **_CACHE.get("run_kwargs", {}))
    _CACHE["last_result"] = res
    ok = np.empty((B, RIMQ), np.float32)
    ov = np.empty((B, VD), np.float32)
    for m in range(NCORES):
        for s in range(BL):
            e = int(order[8 * s + m])
            ok[e] = res.results[m]["out_key"][s]
            ov[e] = res.results[m]["out_val"][s]
    return ok[:, None, :], ov[:, None, :]
